# revision 43
# baseline (speedup 1.0000x reference)
"""DPCA block (dual-pruned cross-attention) Trainium2 kernel, v2.

Sharding: data-parallel over batch. B=8 -> 8 NeuronCores, one batch per core,
weights replicated, zero collectives.

v2 redesign vs baseline:
 - Phase A only mean-subtracts x (f16 x'' tiles); LN rstd is NOT applied to
   x: the k/q paths are scale-invariant (l2norm cancels per-position scale),
   so rstd is applied only to v at il-build time. Variance comes from
   squares of the centered x'' (no E[x^2]-mu^2 chain), and sqrt fuses the
   1/C scale + eps bias into one activation.
 - Stats matmuls on raw f32 x run as f32r (1 cycle/row at >=256 free).
 - Activation-table discipline: phases A+P use Sqrt only, phase X uses Exp
   only, tail swaps back to Sqrt once => 2 table swaps total instead of
   per-chunk thrash.
 - Z normalization: per-chunk Z matvecs land in one [8,512] PSUM bank per
   4-chunk group; one batched reciprocal + one f16 cast serve 4 chunks.
 - Tail: out-LN on centered y, affine fused as (ty*rstd)*gg + qs via
   scalar_tensor_tensor with per-partition gg, residual reload streamed.
"""

import numpy as np

import concourse.bass as bass
import concourse.bacc as bacc
import concourse.mybir as mybir
from concourse.tile import TileContext
from concourse.bass_utils import run_bass_kernel_spmd

F32 = mybir.dt.float32
F32R = mybir.dt.float32r
BF16 = mybir.dt.bfloat16
F16 = mybir.dt.float16
I16 = mybir.dt.int16
I32 = mybir.dt.int32
U32 = mybir.dt.uint32
AX = mybir.AxisListType
OP = mybir.AluOpType
AF = mybir.ActivationFunctionType

C = 256
N = 4096
HEADS = 8
D = 64
PAIRS = 4
INNER = HEADS * D        # 512
NCH = 512
CH = N // NCH            # 8
KEYS = 64                # 8 rows x 8 cols kept per head
EPS = 1e-5


import os
STOP_STAGE = int(os.environ.get("STOP_STAGE", "9"))


def build_program(gval=1.0):
    nc = bacc.Bacc()

    ctx_d = nc.declare_dram_parameter("ctx", [C, N], F32R, False)
    qs_d = nc.declare_dram_parameter("qsrc", [C, N], F32R, False)
    wkvT_d = nc.declare_dram_parameter("wkvT", [C, 2 * INNER], F16, False)
    wqT_d = nc.declare_dram_parameter("wqT", [C, INNER], F16, False)
    woutT_d = nc.declare_dram_parameter("woutT", [INNER, C], BF16, False)
    gg_d = nc.declare_dram_parameter("gg", [C, 1], F32, False)
    ident_d = nc.declare_dram_parameter("identc", [128, 64], F16, False)
    onehot8_d = nc.declare_dram_parameter("onehot8c", [128, 8], F32, False)
    m8f_d = nc.declare_dram_parameter("m8fc", [128, 1], F32, False)
    zsel2_d = nc.declare_dram_parameter("zsel2c", [2, 128], F32, False)
    zsel128_d = nc.declare_dram_parameter("zsel128c", [128, 128], F16, False)
    out_d = nc.declare_dram_parameter("out", [C, N], F32, True)

    with TileContext(nc) as tc:
        with (
            tc.tile_pool(name="const", bufs=1) as constp,
            tc.tile_pool(name="wpool", bufs=1) as wpool,
            tc.tile_pool(name="xin", bufs=3) as xin,
            tc.tile_pool(name="stat", bufs=1) as statp,
            tc.tile_pool(name="xpp", bufs=1) as xpp,
            tc.tile_pool(name="kvq", bufs=1) as kvqp,
            tc.tile_pool(name="pairs", bufs=2) as pairp,
            tc.tile_pool(name="sel", bufs=1) as selp,
            tc.tile_pool(name="attn", bufs=1) as attnp,
            tc.tile_pool(name="ptile", bufs=2) as ptp,
            tc.tile_pool(name="fin", bufs=1) as finp,
            tc.tile_pool(name="psStat", bufs=3, space="PSUM") as psStat,
            tc.tile_pool(name="psMain", bufs=4, space="PSUM") as psMain,
            tc.tile_pool(name="psSmall", bufs=1, space="PSUM") as psSmall,
        ):
            # ------------- constants -------------
            ones128 = constp.tile([128, 128], F32, tag="ones128")
            nc.vector.memset(ones128[:], 1.0)
            ones_r = constp.tile([128, 128], F32R, tag="ones_r")
            nc.scalar.copy(ones_r[:], ones128[:])
            ones16 = constp.tile([128, 128], F16, tag="ones16")
            nc.vector.memset(ones16[:], 1.0)
            ones16b = constp.tile([128, 128], BF16, tag="ones16b")
            nc.vector.memset(ones16b[:], 1.0)
            eps_c = constp.tile([128, 1], F32, tag="eps_c")
            nc.vector.memset(eps_c[:], EPS)
            eps_g = constp.tile([128, 1], F32, tag="eps_g")
            nc.vector.memset(eps_g[:], EPS / (gval * gval))
            # halves8: col 2i ones on partitions 0:64, col 2i+1 on 64:128
            halves8 = constp.tile([128, 8], F16, tag="halves8")
            nc.vector.memset(halves8[:], 0.0)
            for i in range(4):
                nc.vector.memset(halves8[0:64, 2 * i:2 * i + 1], 1.0)
                nc.vector.memset(halves8[64:128, 2 * i + 1:2 * i + 2], 1.0)
            # block-diag ones for per-head-half broadcast sums
            halvesbc16 = constp.tile([128, 128], F16, tag="halvesbc16")
            nc.vector.memset(halvesbc16[:], 0.0)
            nc.vector.memset(halvesbc16[0:64, 0:64], 1.0)
            nc.vector.memset(halvesbc16[64:128, 64:128], 1.0)
            ident16 = constp.tile([128, 64], F16, tag="ident16")
            nc.sync.dma_start(out=ident16[:], in_=ident_d[:])
            zsel2 = constp.tile([2, 128], F32, tag="zsel2")
            nc.sync.dma_start(out=zsel2[:], in_=zsel2_d[:])
            # zsel2f128: selector rows at base 0 and base 64 (matmul requires
            # lhsT/rhs base partitions to match)
            zsel2f128 = constp.tile([128, 128], F16, tag="zsel2f128")
            nc.sync.dma_start(out=zsel2f128[:], in_=zsel128_d[:])
            onehot8 = constp.tile([128, 8], F32, tag="onehot8")
            nc.sync.dma_start(out=onehot8[:], in_=onehot8_d[:])
            m8f = constp.tile([128, 1], F32, tag="m8f")
            nc.sync.dma_start(out=m8f[:], in_=m8f_d[:])

            # ------------- weights -------------
            wkvT = [wpool.tile([128, 2 * INNER], F16, tag=f"wkvT{i}",
                               name=f"wkvT{i}") for i in range(2)]
            wqT = [wpool.tile([128, INNER], F16, tag=f"wqT{i}",
                              name=f"wqT{i}") for i in range(2)]
            for i in range(2):
                nc.sync.dma_start(out=wkvT[i][:], in_=wkvT_d[128 * i:128 * (i + 1), :])
                nc.sync.dma_start(out=wqT[i][:], in_=wqT_d[128 * i:128 * (i + 1), :])
            woutT = [wpool.tile([128, C], BF16, tag=f"woutT{i}",
                                name=f"woutT{i}") for i in range(4)]
            for i in range(4):
                nc.sync.dma_start(out=woutT[i][:], in_=woutT_d[128 * i:128 * (i + 1), :])
            gg = [wpool.tile([128, 1], F32, tag=f"gg{i}", name=f"gg{i}")
                  for i in range(2)]
            for i in range(2):
                nc.sync.dma_start(out=gg[i][:], in_=gg_d[128 * i:128 * (i + 1), :])

            # ------------- phase A: LN (ctx: mu+rstd; qs: mu only) ---------
            # ctx x'' = (x - mu)*rstd (v needs rstd; k's l2norm cancels it);
            # qs x'' = x - mu (q's l2norm cancels any scale). Variance from
            # squares of the centered xc (sqrt fuses 1/C scale + eps bias).
            xpp_t = {
                "ctx": [xpp.tile([128, N], F16, tag=f"xpp_ctx{i}",
                                 name=f"xpp_ctx{i}") for i in range(2)],
                "qs": [xpp.tile([128, N], F16, tag=f"xpp_qs{i}",
                                name=f"xpp_qs{i}") for i in range(2)],
            }

            scopeA = nc.enter_named_scope("phA", False)
            for ch in range(CH):
                sl = slice(ch * NCH, (ch + 1) * NCH)
                xt = [xin.tile([128, NCH], F32R, tag="xt", name="xt", bufs=2)
                      for _ in range(2)]
                xtq = [xin.tile([128, NCH], F32R, tag="xtq", name="xtq", bufs=2)
                       for _ in range(2)]
                for i in range(2):
                    nc.sync.dma_start(out=xt[i][:],
                                      in_=ctx_d[128 * i:128 * (i + 1), sl])
                    nc.scalar.dma_start(out=xtq[i][:],
                                        in_=qs_d[128 * i:128 * (i + 1), sl])
                S_ps = psStat.tile([128, NCH], F32, tag="st")
                nc.tensor.matmul(S_ps[:], lhsT=ones_r[:], rhs=xt[0][:],
                                 start=True, stop=False)
                nc.tensor.matmul(S_ps[:], lhsT=ones_r[:], rhs=xt[1][:],
                                 start=False, stop=True)
                t_mu = statp.tile([128, NCH], F32, tag="t_mu", bufs=2)
                nc.scalar.activation(t_mu[:], S_ps[:], AF.Copy, scale=1.0 / C)
                Sq_ps = psStat.tile([128, NCH], F32, tag="st")
                nc.tensor.matmul(Sq_ps[:], lhsT=ones_r[:], rhs=xtq[0][:],
                                 start=True, stop=False)
                nc.tensor.matmul(Sq_ps[:], lhsT=ones_r[:], rhs=xtq[1][:],
                                 start=False, stop=True)
                t_muq = statp.tile([128, NCH], F32, tag="t_muq", bufs=2)
                nc.scalar.activation(t_muq[:], Sq_ps[:], AF.Copy, scale=1.0 / C)
                # centered ctx (f32, feeds squares + final rstd mult)
                xc = [xin.tile([128, NCH], F32, tag="xc", name="xc", bufs=2)
                      for _ in range(2)]
                for i in range(2):
                    nc.gpsimd.tensor_sub(xc[i][:], xt[i][:].bitcast(F32),
                                         t_mu[:])
                # qs: mean-subtract only
                nc.gpsimd.tensor_sub(xpp_t["qs"][0][:, sl],
                                     xtq[0][:].bitcast(F32), t_muq[:])
                nc.vector.tensor_sub(xpp_t["qs"][1][:, sl],
                                     xtq[1][:].bitcast(F32), t_muq[:])
                # variance of ctx from centered squares
                xsq = [xin.tile([128, NCH], F16, tag="xsq", name="xsq", bufs=2)
                       for _ in range(2)]
                for i in range(2):
                    nc.scalar.activation(xsq[i][:], xc[i][:], AF.Square)
                Q_ps = psStat.tile([128, NCH], F32, tag="st")
                nc.tensor.matmul(Q_ps[:], lhsT=ones16[:], rhs=xsq[0][:],
                                 start=True, stop=False)
                nc.tensor.matmul(Q_ps[:], lhsT=ones16[:], rhs=xsq[1][:],
                                 start=False, stop=True)
                se = statp.tile([128, NCH], F32, tag="se", bufs=2)
                nc.scalar.activation(se[:], Q_ps[:], AF.Sqrt, bias=eps_c[:],
                                     scale=1.0 / C)
                rstd = statp.tile([128, NCH], F32, tag="rstd", bufs=2)
                nc.vector.reciprocal_approx_fast(out=rstd[:], in_=se[:])
                for i in range(2):
                    nc.vector.tensor_tensor(out=xpp_t["ctx"][i][:, sl],
                                            in0=xc[i][:], in1=rstd[:],
                                            op=OP.mult)
            nc.leave_named_scope("phA", scopeA[0], False)

            # ------------- phase P: proj + norms + probe + topk + gather ---
            il_t, qh_t, ksel_t, kbd_t, vbd_t = {}, {}, {}, {}, {}
            kabs_r_t, kabsc8_t = {}, {}
            ao16 = [attnp.tile([128, N], BF16, tag=f"ao{p}", name=f"ao{p}")
                    for p in range(PAIRS)]

            def alloc_pair(p):
                if p not in il_t:
                    il_t[p] = kvqp.tile([128, 2 * N], F16, tag="il", bufs=1,
                                        name=f"il{p}")
                    qh_t[p] = kvqp.tile([128, N], F16, tag="qh", bufs=4,
                                        name=f"qh{p}")

            def do_b1q(p, chunks):
                alloc_pair(p)
                qh = qh_t[p]
                for ch in chunks:
                    sl = slice(ch * NCH, (ch + 1) * NCH)
                    qps = psMain.tile([128, NCH], F32, tag="m")
                    nc.tensor.matmul(qps[:], lhsT=wqT[0][:, 128 * p:128 * (p + 1)],
                                     rhs=xpp_t["qs"][0][:, sl], start=True, stop=False)
                    nc.tensor.matmul(qps[:], lhsT=wqT[1][:, 128 * p:128 * (p + 1)],
                                     rhs=xpp_t["qs"][1][:, sl], start=False, stop=True)
                    q2c = kvqp.tile([128, NCH], F16, tag="q2c", bufs=1)
                    nc.scalar.activation(q2c[:], qps[:], AF.Square)
                    rqps = psStat.tile([128, NCH], F32, tag="st")
                    nc.tensor.matmul(rqps[:], lhsT=halvesbc16[:], rhs=q2c[:],
                                     start=True, stop=True)
                    seq2 = statp.tile([128, NCH], F32, tag="se_", bufs=2)
                    nc.scalar.activation(seq2[:], rqps[:], AF.Sqrt)
                    rbq = statp.tile([128, NCH], F32, tag="rb_", bufs=2)
                    nc.vector.reciprocal_approx_fast(out=rbq[:], in_=seq2[:])
                    nc.vector.tensor_tensor(out=qh[:, sl], in0=qps[:],
                                            in1=rbq[:], op=OP.mult)

            def do_b1kv(p, chunks):
                alloc_pair(p)
                il = il_t[p]
                for ch in chunks:
                    sl = slice(ch * NCH, (ch + 1) * NCH)
                    # --- k projection + l2 factor + il write ---
                    kps = psMain.tile([128, NCH], F32, tag="m")
                    nc.tensor.matmul(kps[:], lhsT=wkvT[0][:, 128 * p:128 * (p + 1)],
                                     rhs=xpp_t["ctx"][0][:, sl], start=True, stop=False)
                    nc.tensor.matmul(kps[:], lhsT=wkvT[1][:, 128 * p:128 * (p + 1)],
                                     rhs=xpp_t["ctx"][1][:, sl], start=False, stop=True)
                    k16c = kvqp.tile([128, NCH], F16, tag="k16c", bufs=2)
                    nc.scalar.copy(k16c[:], kps[:])
                    k2c = kvqp.tile([128, NCH], F16, tag="k2c", bufs=2)
                    nc.gpsimd.tensor_mul(k2c[:], k16c[:], k16c[:])
                    rkps = psStat.tile([128, NCH], F32, tag="st")
                    nc.tensor.matmul(rkps[:], lhsT=halvesbc16[:], rhs=k2c[:],
                                     start=True, stop=True)
                    sek = statp.tile([128, NCH], F32, tag="se_", bufs=2)
                    nc.scalar.activation(sek[:], rkps[:], AF.Sqrt)
                    rbk = statp.tile([128, NCH], F32, tag="rb_", bufs=2)
                    nc.vector.reciprocal_approx_fast(out=rbk[:], in_=sek[:])
                    nc.gpsimd.tensor_tensor(out=il[:, 2 * sl.start:2 * sl.stop:2],
                                            in0=k16c[:], in1=rbk[:], op=OP.mult)
                    # --- v projection (rstd already folded into ctx x'') ---
                    vps = psMain.tile([128, NCH], F32, tag="m")
                    vo = INNER + 128 * p
                    nc.tensor.matmul(vps[:], lhsT=wkvT[0][:, vo:vo + 128],
                                     rhs=xpp_t["ctx"][0][:, sl], start=True, stop=False)
                    nc.tensor.matmul(vps[:], lhsT=wkvT[1][:, vo:vo + 128],
                                     rhs=xpp_t["ctx"][1][:, sl], start=False, stop=True)
                    nc.scalar.copy(il[:, 2 * sl.start + 1:2 * sl.stop:2], vps[:])

            def do_b2(p):
                il, qh = il_t[p], qh_t[p]
                # --- segmented |khat| sums + q_probe + scores + topk ---
                il4 = il[:].rearrange("p (h w d) -> p h w d", h=64, w=64, d=2)
                kabs_r = pairp.tile([128, 64], F32, tag="kabsr")
                nc.vector.tensor_reduce(out=kabs_r[:], in_=il4[:, :, :, 0],
                                        axis=AX.X, op=OP.add, apply_absolute_value=True)
                il4c = il[:].rearrange("p (h w d) -> p w h d", h=64, w=64, d=2)
                kabs_c = pairp.tile([128, 64], F32, tag="kabsc")
                nc.vector.tensor_reduce(out=kabs_c[:], in_=il4c[:, :, :, 0],
                                        axis=AX.X, op=OP.add, apply_absolute_value=True)
                qp = pairp.tile([128, 1], F32, tag="qp")
                nc.vector.tensor_reduce(out=qp[:], in_=qh[:], axis=AX.X, op=OP.add)
                qp2 = pairp.tile([128, 2], F32, tag="qp2")
                nc.vector.memset(qp2[:], 0.0)
                nc.vector.tensor_copy(out=qp2[0:64, 0:1], in_=qp[0:64, :])
                nc.vector.tensor_copy(out=qp2[64:128, 1:2], in_=qp[64:128, :])
                sc_r = pairp.tile([2, 64], F32, tag="scr")
                sc_ps = psSmall.tile([2, 64], F32, tag="s")
                nc.tensor.matmul(sc_ps[:], lhsT=qp2[:], rhs=kabs_r[:],
                                 start=True, stop=True)
                nc.scalar.copy(sc_r[:], sc_ps[:])
                sc_c = pairp.tile([2, 64], F32, tag="scc")
                sc_ps2 = psSmall.tile([2, 64], F32, tag="s")
                nc.tensor.matmul(sc_ps2[:], lhsT=qp2[:], rhs=kabs_c[:],
                                 start=True, stop=True)
                nc.scalar.copy(sc_c[:], sc_ps2[:])
                mx = pairp.tile([2, 8], F32, tag="mx")
                idx_r = pairp.tile([2, 8], U32, tag="idxr")
                nc.vector.max(out=mx[:], in_=sc_r[:])
                nc.vector.max_index(out=idx_r[:], in_max=mx[:], in_values=sc_r[:])
                mxc = pairp.tile([2, 8], F32, tag="mxc")
                idx_c = pairp.tile([2, 8], U32, tag="idxc")
                nc.vector.max(out=mxc[:], in_=sc_c[:])
                nc.vector.max_index(out=idx_c[:], in_max=mxc[:], in_values=sc_c[:])
                idxr_f = pairp.tile([2, 8], F32, tag="idxrf")
                nc.vector.tensor_copy(out=idxr_f[:], in_=idx_r[:])
                idxc_f = pairp.tile([2, 8], F32, tag="idxcf")
                nc.vector.tensor_copy(out=idxc_f[:], in_=idx_c[:])
                # broadcast idx rows to all partitions by head half
                rbc_ps = psSmall.tile([128, 8], F32, tag="s")
                nc.tensor.matmul(rbc_ps[:], lhsT=zsel2[:], rhs=idxr_f[:],
                                 start=True, stop=True)
                rbc = pairp.tile([128, 8], F32, tag="rbc")
                nc.scalar.copy(rbc[:], rbc_ps[:])
                cbc_ps = psSmall.tile([128, 8], F32, tag="s")
                nc.tensor.matmul(cbc_ps[:], lhsT=zsel2[:], rhs=idxc_f[:],
                                 start=True, stop=True)
                cbc = pairp.tile([128, 8], F32, tag="cbc")
                nc.scalar.copy(cbc[:], cbc_ps[:])
                # Bcol[p] = idx_c[h(p), p%8]
                junk8 = pairp.tile([128, 8], F32, tag="junk8")
                nc.vector.tensor_mul(junk8[:], cbc[:], onehot8[:])
                Bcol = pairp.tile([128, 1], F32, tag="Bcol")
                nc.vector.tensor_reduce(out=Bcol[:], in_=junk8[:], axis=AX.X,
                                        op=OP.add)
                # wr[p, s] = idx_r[h(p), 2s + ((p>>3)&1)]
                wdiff = pairp.tile([128, 4], F32, tag="wdiff")
                nc.vector.tensor_sub(wdiff[:], rbc[:, 1:8:2], rbc[:, 0:8:2])
                wsel = pairp.tile([128, 4], F32, tag="wsel")
                nc.vector.tensor_scalar(wsel[:], wdiff[:], m8f[:], scalar2=None,
                                        op0=OP.mult)
                wr = pairp.tile([128, 4], F32, tag="wr")
                nc.vector.tensor_add(wr[:], wsel[:], rbc[:, 0:8:2])
                posfw = pairp.tile([128, 4], F32, tag="posfw")
                nc.vector.scalar_tensor_tensor(out=posfw[:], in0=wr[:], scalar=64.0,
                                               in1=Bcol[:].to_broadcast([128, 4]),
                                               op0=OP.mult, op1=OP.add)
                widx32 = pairp.tile([128, 4], I32, tag="widx32")
                nc.vector.tensor_copy(out=widx32[:], in_=posfw[:])
                widx = pairp.tile([128, 4], I16, tag="widx")
                nc.vector.tensor_copy(out=widx[:], in_=widx32[:])
                # --- gather ---
                ksel_il = selp.tile([128, 128], F16, tag="kselil", bufs=2,
                                    name=f"ksel{p}")
                nc.gpsimd.ap_gather(
                    out_ap=ksel_il[:].rearrange("p (k d) -> p k d", d=2),
                    in_ap=il[:].rearrange("p (n d) -> p n d", d=2),
                    idxs_ap=widx[:],
                    channels=128, num_elems=N, d=2, num_idxs=KEYS)
                ksel_t[p] = ksel_il

            def do_extract(p):
                ksel_il = ksel_t[p]
                kbd = selp.tile([128, 128], F16, tag="kbd", bufs=4, name=f"kbd{p}")
                nc.vector.memset(kbd[:], 0.0)
                nc.vector.tensor_copy(out=kbd[0:64, 0:64], in_=ksel_il[0:64, 0:128:2])
                nc.vector.tensor_copy(out=kbd[64:128, 64:128],
                                      in_=ksel_il[64:128, 0:128:2])
                vbd = selp.tile([128, 128], F16, tag="vbd", bufs=4, name=f"vbd{p}")
                nc.vector.memset(vbd[:], 0.0)
                for h in range(2):
                    o = 64 * h
                    tps = psSmall.tile([64, 64], F16, tag="s")
                    nc.tensor.transpose(out=tps[:], in_=ksel_il[o:o + 64, 1:128:2],
                                        identity=ident16[o:o + 64, :])
                    nc.scalar.copy(vbd[o:o + 64, o:o + 64], tps[:])
                kbd_t[p], vbd_t[p] = kbd, vbd

            # il has a single buffer: gather(p) must be emitted before any
            # il(p+1) writes (in-order gpsimd queue keeps this deadlock-free).
            scopeP = nc.enter_named_scope("phP", False)
            if STOP_STAGE >= 2:
                for p in range(PAIRS):
                    do_b1q(p, range(CH))
                    do_b1kv(p, range(CH))
                    if STOP_STAGE >= 3:
                        do_b2(p)
                        do_extract(p)
            nc.leave_named_scope("phP", scopeP[0], False)

            # ------------- phase X: attention (Exp table) -------------
            scopeX = nc.enter_named_scope("phX", False)

            def do_b3(p):
                kbd, vbd, qh = kbd_t[p], vbd_t[p], qh_t[p]
                for g in range(4):          # 2-chunk groups share one Z bank
                    # chunk 2g -> zall[0:2], chunk 2g+1 -> zall[64:66]
                    zall = psSmall.tile([128, NCH], F32, tag="s",
                                        name=f"zall{p}{g}")
                    pts = []
                    for i in range(2):
                        ch = 2 * g + i
                        sl = slice(ch * NCH, (ch + 1) * NCH)
                        sps = psMain.tile([128, NCH], F32, tag="m")
                        nc.tensor.matmul(sps[:], lhsT=kbd[:], rhs=qh[:, sl],
                                         start=True, stop=True)
                        pt = ptp.tile([128, NCH], F16, tag="pT", bufs=4)
                        nc.scalar.activation(pt[:], sps[:], AF.Exp)
                        nc.tensor.matmul(zall[64 * i:64 * i + 2, :],
                                         lhsT=halves8[:, 0:2],
                                         rhs=pt[:], start=True, stop=True)
                        pts.append(pt)
                    zinv = ptp.tile([128, NCH], F32, tag="zinv", bufs=2)
                    nc.vector.reciprocal_approx_fast(out=zinv[0:66, :],
                                                     in_=zall[0:66, :])
                    zinv16 = ptp.tile([128, NCH], F16, tag="zinv16", bufs=2)
                    nc.scalar.copy(zinv16[0:66, :], zinv[0:66, :])
                    for i in range(2):
                        ch = 2 * g + i
                        sl = slice(ch * NCH, (ch + 1) * NCH)
                        zb = psStat.tile([128, NCH], F32, tag="st")
                        nc.tensor.matmul(zb[:], lhsT=zsel2f128[64 * i:64 * i + 2, :],
                                         rhs=zinv16[64 * i:64 * i + 2, :],
                                         start=True, stop=True)
                        ph16 = ptp.tile([128, NCH], F16, tag="ph16", bufs=2)
                        nc.vector.tensor_tensor(out=ph16[:], in0=pts[i][:],
                                                in1=zb[:], op=OP.mult)
                        pvs = psMain.tile([128, NCH], F32, tag="m")
                        nc.tensor.matmul(pvs[:], lhsT=vbd[:], rhs=ph16[:],
                                         start=True, stop=True)
                        if ch % 2 == 0:
                            nc.scalar.copy(ao16[p][:, sl], pvs[:])
                        else:
                            nc.vector.tensor_copy(out=ao16[p][:, sl], in_=pvs[:])

            if STOP_STAGE >= 4:
                for p in range(PAIRS):
                    do_b3(p)
            nc.leave_named_scope("phX", scopeX[0], False)

            # ------------- tail: out-proj + out-LN + residual -------------
            scopeT = nc.enter_named_scope("phT", False)
            for ch in range(CH if STOP_STAGE >= 5 else 0):
                sl = slice(ch * NCH, (ch + 1) * NCH)
                qs_t = [finp.tile([128, NCH], F32, tag=f"qs_t{i}", bufs=1,
                                  name=f"qs_t{i}_{ch}") for i in range(2)]
                for i in range(2):
                    nc.sync.dma_start(out=qs_t[i][:],
                                      in_=qs_d[128 * i:128 * (i + 1), sl].bitcast(F32))
                y16 = [finp.tile([128, NCH], BF16, tag="y16", name=f"y16_{ch}_{i}",
                                 bufs=2) for i in range(2)]
                for i in range(2):
                    yps = psMain.tile([128, NCH], F32, tag="m")
                    for p in range(PAIRS):
                        nc.tensor.matmul(yps[:],
                                         lhsT=woutT[p][:, 128 * i:128 * (i + 1)],
                                         rhs=ao16[p][:, sl], start=(p == 0),
                                         stop=(p == 3))
                    nc.scalar.copy(y16[i][:, :], yps[:])
                S_ps = psStat.tile([128, NCH], F32, tag="st")
                nc.tensor.matmul(S_ps[:], lhsT=ones16b[:], rhs=y16[0][:],
                                 start=True, stop=False)
                nc.tensor.matmul(S_ps[:], lhsT=ones16b[:], rhs=y16[1][:],
                                 start=False, stop=True)
                t_mu = finp.tile([128, NCH], F32, tag="ft_mu", bufs=1)
                nc.scalar.activation(t_mu[:], S_ps[:], AF.Copy, scale=1.0 / C)
                ty = [finp.tile([128, NCH], BF16, tag="fty", name=f"ty{ch}_{i}",
                                bufs=2) for i in range(2)]
                nc.vector.tensor_sub(ty[0][:], y16[0][:], t_mu[:])
                nc.vector.tensor_sub(ty[1][:], y16[1][:], t_mu[:])
                y2 = [finp.tile([128, NCH], F16, tag="fy2", bufs=1,
                                name=f"y2_{ch}_{i}") for i in range(2)]
                for i in range(2):
                    nc.vector.tensor_mul(y2[i][:], ty[i][:], ty[i][:])
                Q_ps = psStat.tile([128, NCH], F32, tag="st")
                nc.tensor.matmul(Q_ps[:], lhsT=ones16[:], rhs=y2[0][:],
                                 start=True, stop=False)
                nc.tensor.matmul(Q_ps[:], lhsT=ones16[:], rhs=y2[1][:],
                                 start=False, stop=True)
                # sqrt((1/(C*g^2))*x + eps/g^2) = sqrt(x/C+eps)/g, so the
                # reciprocal directly yields g*rstd (g = uniform gamma*on_g).
                se = finp.tile([128, NCH], F32, tag="fse", bufs=1)
                nc.scalar.activation(se[:], Q_ps[:], AF.Sqrt, bias=eps_g[:],
                                     scale=1.0 / (C * gval * gval))
                rstd_o = finp.tile([128, NCH], F32, tag="frstd", bufs=1)
                nc.vector.reciprocal_approx_fast(out=rstd_o[:], in_=se[:])
                for i in range(2):
                    tz = finp.tile([128, NCH], F32, tag="ftz", name=f"tz{ch}_{i}",
                                   bufs=2)
                    nc.vector.tensor_mul(tz[:], ty[i][:], rstd_o[:])
                    ot = finp.tile([128, NCH], F32, tag="fot", name=f"ot{ch}_{i}",
                                   bufs=2)
                    nc.gpsimd.tensor_add(ot[:], tz[:], qs_t[i][:])
                    nc.sync.dma_start(out=out_d[128 * i:128 * (i + 1), sl],
                                      in_=ot[:])
            if STOP_STAGE < 5:
                for ch in range(CH):
                    sl = slice(ch * NCH, (ch + 1) * NCH)
                    for i in range(2):
                        dummy = finp.tile([128, NCH], F32, tag="fot",
                                          name=f"dummy{ch}_{i}", bufs=2)
                        nc.vector.memset(dummy[:], 0.0)
                        nc.sync.dma_start(out=out_d[128 * i:128 * (i + 1), sl],
                                          in_=dummy[:])
            nc.leave_named_scope("phT", scopeT[0], False)
    nc.finalize()
    return nc


_CACHE = {}


def prepare(inputs):
    """Build (nc, in_maps) from full unsharded inputs."""
    qsrc = np.asarray(inputs["query_source"], np.float32)
    ctx = np.asarray(inputs["context"], np.float32)
    cn_g = np.asarray(inputs["cn_g"], np.float32).reshape(C)
    cn_b = np.asarray(inputs["cn_b"], np.float32).reshape(C)
    qn_g = np.asarray(inputs["qn_g"], np.float32).reshape(C)
    qn_b = np.asarray(inputs["qn_b"], np.float32).reshape(C)
    on_g = np.asarray(inputs["on_g"], np.float32).reshape(C)
    on_b = np.asarray(inputs["on_b"], np.float32).reshape(C)
    w_kv = np.asarray(inputs["w_kv"], np.float32)
    w_q = np.asarray(inputs["w_q"], np.float32)
    w_out = np.asarray(inputs["w_out"], np.float32)
    gamma = float(np.asarray(inputs["gamma"], np.float32).reshape(()))

    assert np.abs(cn_b).max() == 0 and np.abs(qn_b).max() == 0 and \
        np.abs(on_b).max() == 0, "nonzero LN bias not implemented"

    import ml_dtypes
    bf16 = ml_dtypes.bfloat16
    wkvT = np.ascontiguousarray((w_kv * cn_g[None, :]).T).astype(np.float16)
    wqT = np.ascontiguousarray((w_q * qn_g[None, :]).T).astype(np.float16)
    woutT = np.ascontiguousarray(w_out.T).astype(bf16)
    gg = np.ascontiguousarray((gamma * on_g).reshape(C, 1), np.float32)

    p_idx = np.arange(128)
    identc = np.zeros((128, 64), np.float16)
    identc[p_idx, p_idx % 64] = 1.0
    onehot8c = (p_idx[:, None] % 8 == np.arange(8)[None, :]).astype(np.float32)
    m8fc = (((p_idx >> 3) & 1).astype(np.float32)).reshape(128, 1)
    zsel2c = (np.arange(128)[None, :] // 64 ==
              np.arange(2)[:, None]).astype(np.float32)
    zsel128c = np.zeros((128, 128), np.float16)
    for b in (0, 64):
        zsel128c[b, 0:64] = 1.0
        zsel128c[b + 1, 64:128] = 1.0

    gv = float(gg[0, 0])
    assert np.allclose(gg, gv), "nonuniform out-LN gain not implemented"
    if _CACHE.get("gval") != gv:
        _CACHE["nc"] = build_program(gv)
        _CACHE["gval"] = gv
    nc = _CACHE["nc"]

    B = qsrc.shape[0]
    in_maps = []
    for b in range(B):
        in_maps.append({
            "ctx": np.ascontiguousarray(ctx[b].reshape(C, N)),
            "qsrc": np.ascontiguousarray(qsrc[b].reshape(C, N)),
            "wkvT": wkvT,
            "wqT": wqT,
            "woutT": woutT,
            "gg": gg,
            "identc": identc,
            "onehot8c": onehot8c,
            "m8fc": m8fc,
            "zsel2c": zsel2c,
            "zsel128c": zsel128c,
        })
    return nc, in_maps


def kernel(**inputs):
    nc, in_maps = prepare(inputs)
    res = run_bass_kernel_spmd(nc, in_maps, core_ids=list(range(8)))
    outs = [np.asarray(r["out"], np.float32).reshape(1, C, 64, 64)
            for r in res.results]
    return np.concatenate(outs, axis=0)


# revision 45
# speedup vs baseline: 1.0123x; 1.0123x over previous
"""DPCA block (dual-pruned cross-attention) Trainium2 kernel, v2.

Sharding: data-parallel over batch. B=8 -> 8 NeuronCores, one batch per core,
weights replicated, zero collectives.

v2 redesign vs baseline:
 - Phase A only mean-subtracts x (f16 x'' tiles); LN rstd is NOT applied to
   x: the k/q paths are scale-invariant (l2norm cancels per-position scale),
   so rstd is applied only to v at il-build time. Variance comes from
   squares of the centered x'' (no E[x^2]-mu^2 chain), and sqrt fuses the
   1/C scale + eps bias into one activation.
 - Stats matmuls on raw f32 x run as f32r (1 cycle/row at >=256 free).
 - Activation-table discipline: phases A+P use Sqrt only, phase X uses Exp
   only, tail swaps back to Sqrt once => 2 table swaps total instead of
   per-chunk thrash.
 - Z normalization: per-chunk Z matvecs land in one [8,512] PSUM bank per
   4-chunk group; one batched reciprocal + one f16 cast serve 4 chunks.
 - Tail: out-LN on centered y, affine fused as (ty*rstd)*gg + qs via
   scalar_tensor_tensor with per-partition gg, residual reload streamed.
"""

import numpy as np

import concourse.bass as bass
import concourse.bacc as bacc
import concourse.mybir as mybir
from concourse.tile import TileContext
from concourse.bass_utils import run_bass_kernel_spmd

F32 = mybir.dt.float32
F32R = mybir.dt.float32r
BF16 = mybir.dt.bfloat16
F16 = mybir.dt.float16
I16 = mybir.dt.int16
I32 = mybir.dt.int32
U32 = mybir.dt.uint32
AX = mybir.AxisListType
OP = mybir.AluOpType
AF = mybir.ActivationFunctionType

C = 256
N = 4096
HEADS = 8
D = 64
PAIRS = 4
INNER = HEADS * D        # 512
NCH = 512
CH = N // NCH            # 8
KEYS = 64                # 8 rows x 8 cols kept per head
EPS = 1e-5


import os
STOP_STAGE = int(os.environ.get("STOP_STAGE", "9"))


def build_program(gval=1.0):
    nc = bacc.Bacc()

    ctx_d = nc.declare_dram_parameter("ctx", [C, N], F32R, False)
    qs_d = nc.declare_dram_parameter("qsrc", [C, N], F32R, False)
    wkvT_d = nc.declare_dram_parameter("wkvT", [C, 2 * INNER], F16, False)
    wqT_d = nc.declare_dram_parameter("wqT", [C, INNER], F16, False)
    woutT_d = nc.declare_dram_parameter("woutT", [INNER, C], BF16, False)
    gg_d = nc.declare_dram_parameter("gg", [C, 1], F32, False)
    ident_d = nc.declare_dram_parameter("identc", [128, 64], F16, False)
    onehot8_d = nc.declare_dram_parameter("onehot8c", [128, 8], F32, False)
    m8f_d = nc.declare_dram_parameter("m8fc", [128, 1], F32, False)
    zsel2_d = nc.declare_dram_parameter("zsel2c", [2, 128], F32, False)
    zsel128_d = nc.declare_dram_parameter("zsel128c", [128, 128], F16, False)
    out_d = nc.declare_dram_parameter("out", [C, N], F32, True)

    with TileContext(nc) as tc:
        with (
            tc.tile_pool(name="const", bufs=1) as constp,
            tc.tile_pool(name="wpool", bufs=1) as wpool,
            tc.tile_pool(name="xin", bufs=3) as xin,
            tc.tile_pool(name="stat", bufs=1) as statp,
            tc.tile_pool(name="xpp", bufs=1) as xpp,
            tc.tile_pool(name="kvq", bufs=1) as kvqp,
            tc.tile_pool(name="pairs", bufs=2) as pairp,
            tc.tile_pool(name="sel", bufs=1) as selp,
            tc.tile_pool(name="attn", bufs=1) as attnp,
            tc.tile_pool(name="ptile", bufs=2) as ptp,
            tc.tile_pool(name="fin", bufs=1) as finp,
            tc.tile_pool(name="psStat", bufs=3, space="PSUM") as psStat,
            tc.tile_pool(name="psMain", bufs=4, space="PSUM") as psMain,
            tc.tile_pool(name="psSmall", bufs=1, space="PSUM") as psSmall,
        ):
            # ------------- constants -------------
            ones128 = constp.tile([128, 128], F32, tag="ones128")
            nc.vector.memset(ones128[:], 1.0)
            ones_r = constp.tile([128, 128], F32R, tag="ones_r")
            nc.scalar.copy(ones_r[:], ones128[:])
            ones16 = constp.tile([128, 128], F16, tag="ones16")
            nc.vector.memset(ones16[:], 1.0)
            ones16b = constp.tile([128, 128], BF16, tag="ones16b")
            nc.vector.memset(ones16b[:], 1.0)
            eps_c = constp.tile([128, 1], F32, tag="eps_c")
            nc.vector.memset(eps_c[:], EPS)
            eps_g = constp.tile([128, 1], F32, tag="eps_g")
            nc.vector.memset(eps_g[:], EPS / (gval * gval))
            # halves8: col 2i ones on partitions 0:64, col 2i+1 on 64:128
            halves8 = constp.tile([128, 8], F16, tag="halves8")
            nc.vector.memset(halves8[:], 0.0)
            for i in range(4):
                nc.vector.memset(halves8[0:64, 2 * i:2 * i + 1], 1.0)
                nc.vector.memset(halves8[64:128, 2 * i + 1:2 * i + 2], 1.0)
            # block-diag ones for per-head-half broadcast sums
            halvesbc16 = constp.tile([128, 128], F16, tag="halvesbc16")
            nc.vector.memset(halvesbc16[:], 0.0)
            nc.vector.memset(halvesbc16[0:64, 0:64], 1.0)
            nc.vector.memset(halvesbc16[64:128, 64:128], 1.0)
            ident16 = constp.tile([128, 64], F16, tag="ident16")
            nc.sync.dma_start(out=ident16[:], in_=ident_d[:])
            zsel2 = constp.tile([2, 128], F32, tag="zsel2")
            nc.sync.dma_start(out=zsel2[:], in_=zsel2_d[:])
            # zsel2f128: selector rows at base 0 and base 64 (matmul requires
            # lhsT/rhs base partitions to match)
            zsel2f128 = constp.tile([128, 128], F16, tag="zsel2f128")
            nc.sync.dma_start(out=zsel2f128[:], in_=zsel128_d[:])
            onehot8 = constp.tile([128, 8], F32, tag="onehot8")
            nc.sync.dma_start(out=onehot8[:], in_=onehot8_d[:])
            m8f = constp.tile([128, 1], F32, tag="m8f")
            nc.sync.dma_start(out=m8f[:], in_=m8f_d[:])

            # ------------- weights -------------
            wkvT = [wpool.tile([128, 2 * INNER], F16, tag=f"wkvT{i}",
                               name=f"wkvT{i}") for i in range(2)]
            wqT = [wpool.tile([128, INNER], F16, tag=f"wqT{i}",
                              name=f"wqT{i}") for i in range(2)]
            for i in range(2):
                nc.sync.dma_start(out=wkvT[i][:], in_=wkvT_d[128 * i:128 * (i + 1), :])
                nc.sync.dma_start(out=wqT[i][:], in_=wqT_d[128 * i:128 * (i + 1), :])
            woutT = [wpool.tile([128, C], BF16, tag=f"woutT{i}",
                                name=f"woutT{i}") for i in range(4)]
            for i in range(4):
                nc.sync.dma_start(out=woutT[i][:], in_=woutT_d[128 * i:128 * (i + 1), :])
            gg = [wpool.tile([128, 1], F32, tag=f"gg{i}", name=f"gg{i}")
                  for i in range(2)]
            for i in range(2):
                nc.sync.dma_start(out=gg[i][:], in_=gg_d[128 * i:128 * (i + 1), :])

            # ------------- phase A: LN (ctx: mu+rstd; qs: mu only) ---------
            # ctx x'' = (x - mu)*rstd (v needs rstd; k's l2norm cancels it);
            # qs x'' = x - mu (q's l2norm cancels any scale). Variance from
            # squares of the centered xc (sqrt fuses 1/C scale + eps bias).
            xpp_t = {
                "ctx": [xpp.tile([128, N], F16, tag=f"xpp_ctx{i}",
                                 name=f"xpp_ctx{i}") for i in range(2)],
                "qs": [xpp.tile([128, N], F16, tag=f"xpp_qs{i}",
                                name=f"xpp_qs{i}") for i in range(2)],
            }

            scopeA = nc.enter_named_scope("phA", False)
            for ch in range(CH):
                sl = slice(ch * NCH, (ch + 1) * NCH)
                xt = [xin.tile([128, NCH], F32R, tag="xt", name="xt", bufs=2)
                      for _ in range(2)]
                xtq = [xin.tile([128, NCH], F32R, tag="xtq", name="xtq", bufs=2)
                       for _ in range(2)]
                for i in range(2):
                    nc.sync.dma_start(out=xt[i][:],
                                      in_=ctx_d[128 * i:128 * (i + 1), sl])
                    nc.scalar.dma_start(out=xtq[i][:],
                                        in_=qs_d[128 * i:128 * (i + 1), sl])
                S_ps = psStat.tile([128, NCH], F32, tag="st")
                nc.tensor.matmul(S_ps[:], lhsT=ones_r[:], rhs=xt[0][:],
                                 start=True, stop=False)
                nc.tensor.matmul(S_ps[:], lhsT=ones_r[:], rhs=xt[1][:],
                                 start=False, stop=True)
                t_mu = statp.tile([128, NCH], F32, tag="t_mu", bufs=2)
                nc.scalar.activation(t_mu[:], S_ps[:], AF.Copy, scale=1.0 / C)
                Sq_ps = psStat.tile([128, NCH], F32, tag="st")
                nc.tensor.matmul(Sq_ps[:], lhsT=ones_r[:], rhs=xtq[0][:],
                                 start=True, stop=False)
                nc.tensor.matmul(Sq_ps[:], lhsT=ones_r[:], rhs=xtq[1][:],
                                 start=False, stop=True)
                t_muq = statp.tile([128, NCH], F32, tag="t_muq", bufs=2)
                nc.scalar.activation(t_muq[:], Sq_ps[:], AF.Copy, scale=1.0 / C)
                # centered ctx (f32, feeds squares + final rstd mult)
                xc = [xin.tile([128, NCH], F32, tag="xc", name="xc", bufs=2)
                      for _ in range(2)]
                for i in range(2):
                    nc.gpsimd.tensor_sub(xc[i][:], xt[i][:].bitcast(F32),
                                         t_mu[:])
                # qs: mean-subtract only
                nc.gpsimd.tensor_sub(xpp_t["qs"][0][:, sl],
                                     xtq[0][:].bitcast(F32), t_muq[:])
                nc.vector.tensor_sub(xpp_t["qs"][1][:, sl],
                                     xtq[1][:].bitcast(F32), t_muq[:])
                # variance of ctx from centered squares
                xsq = [xin.tile([128, NCH], F16, tag="xsq", name="xsq", bufs=2)
                       for _ in range(2)]
                for i in range(2):
                    nc.scalar.activation(xsq[i][:], xc[i][:], AF.Square)
                Q_ps = psStat.tile([128, NCH], F32, tag="st")
                nc.tensor.matmul(Q_ps[:], lhsT=ones16[:], rhs=xsq[0][:],
                                 start=True, stop=False)
                nc.tensor.matmul(Q_ps[:], lhsT=ones16[:], rhs=xsq[1][:],
                                 start=False, stop=True)
                se = statp.tile([128, NCH], F32, tag="se", bufs=2)
                nc.scalar.activation(se[:], Q_ps[:], AF.Sqrt, bias=eps_c[:],
                                     scale=1.0 / C)
                rstd = statp.tile([128, NCH], F32, tag="rstd", bufs=2)
                nc.vector.reciprocal_approx_fast(out=rstd[:], in_=se[:])
                for i in range(2):
                    nc.vector.tensor_tensor(out=xpp_t["ctx"][i][:, sl],
                                            in0=xc[i][:], in1=rstd[:],
                                            op=OP.mult)
            nc.leave_named_scope("phA", scopeA[0], False)

            # ------------- phase P: proj + norms + probe + topk + gather ---
            il_t, qh_t, ksel_t, kbd_t, vbd_t = {}, {}, {}, {}, {}
            kabs_r_t, kabs_c_t, qp8_t = {}, {}, {}
            ao16 = [attnp.tile([128, N], BF16, tag=f"ao{p}", name=f"ao{p}")
                    for p in range(PAIRS)]

            def alloc_pair(p):
                if p not in il_t:
                    il_t[p] = kvqp.tile([128, 2 * N], F16, tag="il", bufs=1,
                                        name=f"il{p}")
                    qh_t[p] = kvqp.tile([128, N], F16, tag="qh", bufs=4,
                                        name=f"qh{p}")
                    kabs_r_t[p] = pairp.tile([128, 64], F32, tag="kabsr",
                                             name=f"kabsr{p}")
                    kabs_c_t[p] = pairp.tile([128, 64], F32, tag="kabsc",
                                             name=f"kabsc{p}")
                    qp8_t[p] = pairp.tile([128, 8], F32, tag="qp8",
                                          name=f"qp8{p}")

            def do_b1q(p, chunks):
                alloc_pair(p)
                qh = qh_t[p]
                for ch in chunks:
                    sl = slice(ch * NCH, (ch + 1) * NCH)
                    qps = psMain.tile([128, NCH], F32, tag="m")
                    nc.tensor.matmul(qps[:], lhsT=wqT[0][:, 128 * p:128 * (p + 1)],
                                     rhs=xpp_t["qs"][0][:, sl], start=True, stop=False)
                    nc.tensor.matmul(qps[:], lhsT=wqT[1][:, 128 * p:128 * (p + 1)],
                                     rhs=xpp_t["qs"][1][:, sl], start=False, stop=True)
                    q2c = kvqp.tile([128, NCH], F16, tag="q2c", bufs=1)
                    nc.scalar.activation(q2c[:], qps[:], AF.Square)
                    rqps = psStat.tile([128, NCH], F32, tag="st")
                    nc.tensor.matmul(rqps[:], lhsT=halvesbc16[:], rhs=q2c[:],
                                     start=True, stop=True)
                    seq2 = statp.tile([128, NCH], F32, tag="se_", bufs=2)
                    nc.scalar.activation(seq2[:], rqps[:], AF.Sqrt)
                    rbq = statp.tile([128, NCH], F32, tag="rb_", bufs=2)
                    nc.vector.reciprocal_approx_fast(out=rbq[:], in_=seq2[:])
                    nc.vector.tensor_tensor(out=qh[:, sl], in0=qps[:],
                                            in1=rbq[:], op=OP.mult)
                    nc.vector.tensor_reduce(out=qp8_t[p][:, ch:ch + 1],
                                            in_=qh[:, sl], axis=AX.X, op=OP.add)

            def do_b1kv(p, chunks):
                alloc_pair(p)
                il = il_t[p]
                for ch in chunks:
                    sl = slice(ch * NCH, (ch + 1) * NCH)
                    # --- k projection + l2 factor + il write ---
                    kps = psMain.tile([128, NCH], F32, tag="m")
                    nc.tensor.matmul(kps[:], lhsT=wkvT[0][:, 128 * p:128 * (p + 1)],
                                     rhs=xpp_t["ctx"][0][:, sl], start=True, stop=False)
                    nc.tensor.matmul(kps[:], lhsT=wkvT[1][:, 128 * p:128 * (p + 1)],
                                     rhs=xpp_t["ctx"][1][:, sl], start=False, stop=True)
                    k16c = kvqp.tile([128, NCH], F16, tag="k16c", bufs=2)
                    nc.scalar.copy(k16c[:], kps[:])
                    k2c = kvqp.tile([128, NCH], F16, tag="k2c", bufs=2)
                    nc.vector.tensor_mul(k2c[:], k16c[:], k16c[:])
                    rkps = psStat.tile([128, NCH], F32, tag="st")
                    nc.tensor.matmul(rkps[:], lhsT=halvesbc16[:], rhs=k2c[:],
                                     start=True, stop=True)
                    sek = statp.tile([128, NCH], F32, tag="se_", bufs=2)
                    nc.scalar.activation(sek[:], rkps[:], AF.Sqrt)
                    rbk = statp.tile([128, NCH], F32, tag="rb_", bufs=2)
                    nc.vector.reciprocal_approx_fast(out=rbk[:], in_=sek[:])
                    nc.gpsimd.tensor_tensor(out=il[:, 2 * sl.start:2 * sl.stop:2],
                                            in0=k16c[:], in1=rbk[:], op=OP.mult)
                    ilc = il[:, 2 * sl.start:2 * sl.stop:2]
                    ilr3 = ilc.rearrange("p (h w) -> p h w", h=8, w=64)
                    nc.vector.tensor_reduce(out=kabs_r_t[p][:, 8 * ch:8 * ch + 8],
                                            in_=ilr3, axis=AX.X, op=OP.add,
                                            apply_absolute_value=True)
                    ilc3 = ilc.rearrange("p (h w) -> p w h", h=8, w=64)
                    if ch == 0:
                        nc.vector.tensor_reduce(out=kabs_c_t[p][:],
                                                in_=ilc3, axis=AX.X, op=OP.add,
                                                apply_absolute_value=True)
                    else:
                        tmpc = pairp.tile([128, 64], F32, tag="tmpc", bufs=2)
                        nc.vector.tensor_reduce(out=tmpc[:], in_=ilc3,
                                                axis=AX.X, op=OP.add,
                                                apply_absolute_value=True)
                        nc.vector.tensor_add(kabs_c_t[p][:], kabs_c_t[p][:],
                                             tmpc[:])
                    # --- v projection (rstd already folded into ctx x'') ---
                    vps = psMain.tile([128, NCH], F32, tag="m")
                    vo = INNER + 128 * p
                    nc.tensor.matmul(vps[:], lhsT=wkvT[0][:, vo:vo + 128],
                                     rhs=xpp_t["ctx"][0][:, sl], start=True, stop=False)
                    nc.tensor.matmul(vps[:], lhsT=wkvT[1][:, vo:vo + 128],
                                     rhs=xpp_t["ctx"][1][:, sl], start=False, stop=True)
                    nc.scalar.copy(il[:, 2 * sl.start + 1:2 * sl.stop:2], vps[:])

            def do_b2(p):
                il, qh = il_t[p], qh_t[p]
                # --- probe partials already accumulated in b1 ---
                kabs_r, kabs_c = kabs_r_t[p], kabs_c_t[p]
                qp = pairp.tile([128, 1], F32, tag="qp")
                nc.vector.tensor_reduce(out=qp[:], in_=qp8_t[p][:], axis=AX.X,
                                        op=OP.add)
                qp2 = pairp.tile([128, 2], F32, tag="qp2")
                nc.vector.memset(qp2[:], 0.0)
                nc.vector.tensor_copy(out=qp2[0:64, 0:1], in_=qp[0:64, :])
                nc.vector.tensor_copy(out=qp2[64:128, 1:2], in_=qp[64:128, :])
                sc_r = pairp.tile([2, 64], F32, tag="scr")
                sc_ps = psSmall.tile([2, 64], F32, tag="s")
                nc.tensor.matmul(sc_ps[:], lhsT=qp2[:], rhs=kabs_r[:],
                                 start=True, stop=True)
                nc.scalar.copy(sc_r[:], sc_ps[:])
                sc_c = pairp.tile([2, 64], F32, tag="scc")
                sc_ps2 = psSmall.tile([2, 64], F32, tag="s")
                nc.tensor.matmul(sc_ps2[:], lhsT=qp2[:], rhs=kabs_c[:],
                                 start=True, stop=True)
                nc.scalar.copy(sc_c[:], sc_ps2[:])
                mx = pairp.tile([2, 8], F32, tag="mx")
                idx_r = pairp.tile([2, 8], U32, tag="idxr")
                nc.vector.max(out=mx[:], in_=sc_r[:])
                nc.vector.max_index(out=idx_r[:], in_max=mx[:], in_values=sc_r[:])
                mxc = pairp.tile([2, 8], F32, tag="mxc")
                idx_c = pairp.tile([2, 8], U32, tag="idxc")
                nc.vector.max(out=mxc[:], in_=sc_c[:])
                nc.vector.max_index(out=idx_c[:], in_max=mxc[:], in_values=sc_c[:])
                idxr_f = pairp.tile([2, 8], F32, tag="idxrf")
                nc.vector.tensor_copy(out=idxr_f[:], in_=idx_r[:])
                idxc_f = pairp.tile([2, 8], F32, tag="idxcf")
                nc.vector.tensor_copy(out=idxc_f[:], in_=idx_c[:])
                # broadcast idx rows to all partitions by head half
                rbc_ps = psSmall.tile([128, 8], F32, tag="s")
                nc.tensor.matmul(rbc_ps[:], lhsT=zsel2[:], rhs=idxr_f[:],
                                 start=True, stop=True)
                rbc = pairp.tile([128, 8], F32, tag="rbc")
                nc.scalar.copy(rbc[:], rbc_ps[:])
                cbc_ps = psSmall.tile([128, 8], F32, tag="s")
                nc.tensor.matmul(cbc_ps[:], lhsT=zsel2[:], rhs=idxc_f[:],
                                 start=True, stop=True)
                cbc = pairp.tile([128, 8], F32, tag="cbc")
                nc.scalar.copy(cbc[:], cbc_ps[:])
                # Bcol[p] = idx_c[h(p), p%8]
                junk8 = pairp.tile([128, 8], F32, tag="junk8")
                nc.vector.tensor_mul(junk8[:], cbc[:], onehot8[:])
                Bcol = pairp.tile([128, 1], F32, tag="Bcol")
                nc.vector.tensor_reduce(out=Bcol[:], in_=junk8[:], axis=AX.X,
                                        op=OP.add)
                # wr[p, s] = idx_r[h(p), 2s + ((p>>3)&1)]
                wdiff = pairp.tile([128, 4], F32, tag="wdiff")
                nc.vector.tensor_sub(wdiff[:], rbc[:, 1:8:2], rbc[:, 0:8:2])
                wsel = pairp.tile([128, 4], F32, tag="wsel")
                nc.vector.tensor_scalar(wsel[:], wdiff[:], m8f[:], scalar2=None,
                                        op0=OP.mult)
                wr = pairp.tile([128, 4], F32, tag="wr")
                nc.vector.tensor_add(wr[:], wsel[:], rbc[:, 0:8:2])
                posfw = pairp.tile([128, 4], F32, tag="posfw")
                nc.vector.scalar_tensor_tensor(out=posfw[:], in0=wr[:], scalar=64.0,
                                               in1=Bcol[:].to_broadcast([128, 4]),
                                               op0=OP.mult, op1=OP.add)
                widx32 = pairp.tile([128, 4], I32, tag="widx32")
                nc.vector.tensor_copy(out=widx32[:], in_=posfw[:])
                widx = pairp.tile([128, 4], I16, tag="widx")
                nc.vector.tensor_copy(out=widx[:], in_=widx32[:])
                # --- gather ---
                ksel_il = selp.tile([128, 128], F16, tag="kselil", bufs=2,
                                    name=f"ksel{p}")
                nc.gpsimd.ap_gather(
                    out_ap=ksel_il[:].rearrange("p (k d) -> p k d", d=2),
                    in_ap=il[:].rearrange("p (n d) -> p n d", d=2),
                    idxs_ap=widx[:],
                    channels=128, num_elems=N, d=2, num_idxs=KEYS)
                ksel_t[p] = ksel_il

            def do_extract(p):
                ksel_il = ksel_t[p]
                kbd = selp.tile([128, 128], F16, tag="kbd", bufs=4, name=f"kbd{p}")
                nc.vector.memset(kbd[:], 0.0)
                nc.vector.tensor_copy(out=kbd[0:64, 0:64], in_=ksel_il[0:64, 0:128:2])
                nc.vector.tensor_copy(out=kbd[64:128, 64:128],
                                      in_=ksel_il[64:128, 0:128:2])
                vbd = selp.tile([128, 128], F16, tag="vbd", bufs=4, name=f"vbd{p}")
                nc.vector.memset(vbd[:], 0.0)
                for h in range(2):
                    o = 64 * h
                    tps = psSmall.tile([64, 64], F16, tag="s")
                    nc.tensor.transpose(out=tps[:], in_=ksel_il[o:o + 64, 1:128:2],
                                        identity=ident16[o:o + 64, :])
                    nc.scalar.copy(vbd[o:o + 64, o:o + 64], tps[:])
                kbd_t[p], vbd_t[p] = kbd, vbd

            # il has a single buffer: gather(p) must be emitted before any
            # il(p+1) writes (in-order gpsimd queue keeps this deadlock-free).
            scopeP = nc.enter_named_scope("phP", False)
            if STOP_STAGE >= 2:
                for p in range(PAIRS):
                    do_b1q(p, range(CH))
                    do_b1kv(p, range(CH))
                    if STOP_STAGE >= 3:
                        do_b2(p)
                        do_extract(p)
            nc.leave_named_scope("phP", scopeP[0], False)

            # ------------- phase X: attention (Exp table) -------------
            scopeX = nc.enter_named_scope("phX", False)

            def do_b3(p):
                kbd, vbd, qh = kbd_t[p], vbd_t[p], qh_t[p]
                for g in range(4):          # 2-chunk groups share one Z bank
                    # chunk 2g -> zall[0:2], chunk 2g+1 -> zall[64:66]
                    zall = psSmall.tile([128, NCH], F32, tag="s",
                                        name=f"zall{p}{g}")
                    pts = []
                    for i in range(2):
                        ch = 2 * g + i
                        sl = slice(ch * NCH, (ch + 1) * NCH)
                        sps = psMain.tile([128, NCH], F32, tag="m")
                        nc.tensor.matmul(sps[:], lhsT=kbd[:], rhs=qh[:, sl],
                                         start=True, stop=True)
                        pt = ptp.tile([128, NCH], F16, tag="pT", bufs=4)
                        nc.scalar.activation(pt[:], sps[:], AF.Exp)
                        nc.tensor.matmul(zall[64 * i:64 * i + 2, :],
                                         lhsT=halves8[:, 0:2],
                                         rhs=pt[:], start=True, stop=True)
                        pts.append(pt)
                    zinv = ptp.tile([128, NCH], F32, tag="zinv", bufs=2)
                    nc.vector.reciprocal_approx_fast(out=zinv[0:66, :],
                                                     in_=zall[0:66, :])
                    zinv16 = ptp.tile([128, NCH], F16, tag="zinv16", bufs=2)
                    nc.scalar.copy(zinv16[0:66, :], zinv[0:66, :])
                    for i in range(2):
                        ch = 2 * g + i
                        sl = slice(ch * NCH, (ch + 1) * NCH)
                        zb = psStat.tile([128, NCH], F32, tag="st")
                        nc.tensor.matmul(zb[:], lhsT=zsel2f128[64 * i:64 * i + 2, :],
                                         rhs=zinv16[64 * i:64 * i + 2, :],
                                         start=True, stop=True)
                        ph16 = ptp.tile([128, NCH], F16, tag="ph16", bufs=2)
                        nc.vector.tensor_tensor(out=ph16[:], in0=pts[i][:],
                                                in1=zb[:], op=OP.mult)
                        pvs = psMain.tile([128, NCH], F32, tag="m")
                        nc.tensor.matmul(pvs[:], lhsT=vbd[:], rhs=ph16[:],
                                         start=True, stop=True)
                        if ch % 2 == 0:
                            nc.scalar.copy(ao16[p][:, sl], pvs[:])
                        else:
                            nc.vector.tensor_copy(out=ao16[p][:, sl], in_=pvs[:])

            if STOP_STAGE >= 4:
                for p in range(PAIRS):
                    do_b3(p)
            nc.leave_named_scope("phX", scopeX[0], False)

            # ------------- tail: out-proj + out-LN + residual -------------
            scopeT = nc.enter_named_scope("phT", False)
            for ch in range(CH if STOP_STAGE >= 5 else 0):
                sl = slice(ch * NCH, (ch + 1) * NCH)
                qs_t = [finp.tile([128, NCH], F32, tag=f"qs_t{i}", bufs=1,
                                  name=f"qs_t{i}_{ch}") for i in range(2)]
                for i in range(2):
                    nc.sync.dma_start(out=qs_t[i][:],
                                      in_=qs_d[128 * i:128 * (i + 1), sl].bitcast(F32))
                y16 = [finp.tile([128, NCH], BF16, tag="y16", name=f"y16_{ch}_{i}",
                                 bufs=2) for i in range(2)]
                for i in range(2):
                    yps = psMain.tile([128, NCH], F32, tag="m")
                    for p in range(PAIRS):
                        nc.tensor.matmul(yps[:],
                                         lhsT=woutT[p][:, 128 * i:128 * (i + 1)],
                                         rhs=ao16[p][:, sl], start=(p == 0),
                                         stop=(p == 3))
                    nc.scalar.copy(y16[i][:, :], yps[:])
                S_ps = psStat.tile([128, NCH], F32, tag="st")
                nc.tensor.matmul(S_ps[:], lhsT=ones16b[:], rhs=y16[0][:],
                                 start=True, stop=False)
                nc.tensor.matmul(S_ps[:], lhsT=ones16b[:], rhs=y16[1][:],
                                 start=False, stop=True)
                t_mu = finp.tile([128, NCH], F32, tag="ft_mu", bufs=1)
                nc.scalar.activation(t_mu[:], S_ps[:], AF.Copy, scale=1.0 / C)
                ty = [finp.tile([128, NCH], BF16, tag="fty", name=f"ty{ch}_{i}",
                                bufs=2) for i in range(2)]
                nc.vector.tensor_sub(ty[0][:], y16[0][:], t_mu[:])
                nc.vector.tensor_sub(ty[1][:], y16[1][:], t_mu[:])
                y2 = [finp.tile([128, NCH], F16, tag="fy2", bufs=1,
                                name=f"y2_{ch}_{i}") for i in range(2)]
                for i in range(2):
                    nc.vector.tensor_mul(y2[i][:], ty[i][:], ty[i][:])
                Q_ps = psStat.tile([128, NCH], F32, tag="st")
                nc.tensor.matmul(Q_ps[:], lhsT=ones16[:], rhs=y2[0][:],
                                 start=True, stop=False)
                nc.tensor.matmul(Q_ps[:], lhsT=ones16[:], rhs=y2[1][:],
                                 start=False, stop=True)
                # sqrt((1/(C*g^2))*x + eps/g^2) = sqrt(x/C+eps)/g, so the
                # reciprocal directly yields g*rstd (g = uniform gamma*on_g).
                se = finp.tile([128, NCH], F32, tag="fse", bufs=1)
                nc.scalar.activation(se[:], Q_ps[:], AF.Sqrt, bias=eps_g[:],
                                     scale=1.0 / (C * gval * gval))
                rstd_o = finp.tile([128, NCH], F32, tag="frstd", bufs=1)
                nc.vector.reciprocal_approx_fast(out=rstd_o[:], in_=se[:])
                for i in range(2):
                    tz = finp.tile([128, NCH], F32, tag="ftz", name=f"tz{ch}_{i}",
                                   bufs=2)
                    nc.vector.tensor_mul(tz[:], ty[i][:], rstd_o[:])
                    ot = finp.tile([128, NCH], F32, tag="fot", name=f"ot{ch}_{i}",
                                   bufs=2)
                    nc.gpsimd.tensor_add(ot[:], tz[:], qs_t[i][:])
                    nc.sync.dma_start(out=out_d[128 * i:128 * (i + 1), sl],
                                      in_=ot[:])
            if STOP_STAGE < 5:
                for ch in range(CH):
                    sl = slice(ch * NCH, (ch + 1) * NCH)
                    for i in range(2):
                        dummy = finp.tile([128, NCH], F32, tag="fot",
                                          name=f"dummy{ch}_{i}", bufs=2)
                        nc.vector.memset(dummy[:], 0.0)
                        nc.sync.dma_start(out=out_d[128 * i:128 * (i + 1), sl],
                                          in_=dummy[:])
            nc.leave_named_scope("phT", scopeT[0], False)
    nc.finalize()
    return nc


_CACHE = {}


def prepare(inputs):
    """Build (nc, in_maps) from full unsharded inputs."""
    qsrc = np.asarray(inputs["query_source"], np.float32)
    ctx = np.asarray(inputs["context"], np.float32)
    cn_g = np.asarray(inputs["cn_g"], np.float32).reshape(C)
    cn_b = np.asarray(inputs["cn_b"], np.float32).reshape(C)
    qn_g = np.asarray(inputs["qn_g"], np.float32).reshape(C)
    qn_b = np.asarray(inputs["qn_b"], np.float32).reshape(C)
    on_g = np.asarray(inputs["on_g"], np.float32).reshape(C)
    on_b = np.asarray(inputs["on_b"], np.float32).reshape(C)
    w_kv = np.asarray(inputs["w_kv"], np.float32)
    w_q = np.asarray(inputs["w_q"], np.float32)
    w_out = np.asarray(inputs["w_out"], np.float32)
    gamma = float(np.asarray(inputs["gamma"], np.float32).reshape(()))

    assert np.abs(cn_b).max() == 0 and np.abs(qn_b).max() == 0 and \
        np.abs(on_b).max() == 0, "nonzero LN bias not implemented"

    import ml_dtypes
    bf16 = ml_dtypes.bfloat16
    wkvT = np.ascontiguousarray((w_kv * cn_g[None, :]).T).astype(np.float16)
    wqT = np.ascontiguousarray((w_q * qn_g[None, :]).T).astype(np.float16)
    woutT = np.ascontiguousarray(w_out.T).astype(bf16)
    gg = np.ascontiguousarray((gamma * on_g).reshape(C, 1), np.float32)

    p_idx = np.arange(128)
    identc = np.zeros((128, 64), np.float16)
    identc[p_idx, p_idx % 64] = 1.0
    onehot8c = (p_idx[:, None] % 8 == np.arange(8)[None, :]).astype(np.float32)
    m8fc = (((p_idx >> 3) & 1).astype(np.float32)).reshape(128, 1)
    zsel2c = (np.arange(128)[None, :] // 64 ==
              np.arange(2)[:, None]).astype(np.float32)
    zsel128c = np.zeros((128, 128), np.float16)
    for b in (0, 64):
        zsel128c[b, 0:64] = 1.0
        zsel128c[b + 1, 64:128] = 1.0

    gv = float(gg[0, 0])
    assert np.allclose(gg, gv), "nonuniform out-LN gain not implemented"
    if _CACHE.get("gval") != gv:
        _CACHE["nc"] = build_program(gv)
        _CACHE["gval"] = gv
    nc = _CACHE["nc"]

    B = qsrc.shape[0]
    in_maps = []
    for b in range(B):
        in_maps.append({
            "ctx": np.ascontiguousarray(ctx[b].reshape(C, N)),
            "qsrc": np.ascontiguousarray(qsrc[b].reshape(C, N)),
            "wkvT": wkvT,
            "wqT": wqT,
            "woutT": woutT,
            "gg": gg,
            "identc": identc,
            "onehot8c": onehot8c,
            "m8fc": m8fc,
            "zsel2c": zsel2c,
            "zsel128c": zsel128c,
        })
    return nc, in_maps


def kernel(**inputs):
    nc, in_maps = prepare(inputs)
    res = run_bass_kernel_spmd(nc, in_maps, core_ids=list(range(8)))
    outs = [np.asarray(r["out"], np.float32).reshape(1, C, 64, 64)
            for r in res.results]
    return np.concatenate(outs, axis=0)


# revision 47
# speedup vs baseline: 1.0423x; 1.0296x over previous
"""DPCA block (dual-pruned cross-attention) Trainium2 kernel, v2.

Sharding: data-parallel over batch. B=8 -> 8 NeuronCores, one batch per core,
weights replicated, zero collectives.

v2 redesign vs baseline:
 - Phase A only mean-subtracts x (f16 x'' tiles); LN rstd is NOT applied to
   x: the k/q paths are scale-invariant (l2norm cancels per-position scale),
   so rstd is applied only to v at il-build time. Variance comes from
   squares of the centered x'' (no E[x^2]-mu^2 chain), and sqrt fuses the
   1/C scale + eps bias into one activation.
 - Stats matmuls on raw f32 x run as f32r (1 cycle/row at >=256 free).
 - Activation-table discipline: phases A+P use Sqrt only, phase X uses Exp
   only, tail swaps back to Sqrt once => 2 table swaps total instead of
   per-chunk thrash.
 - Z normalization: per-chunk Z matvecs land in one [8,512] PSUM bank per
   4-chunk group; one batched reciprocal + one f16 cast serve 4 chunks.
 - Tail: out-LN on centered y, affine fused as (ty*rstd)*gg + qs via
   scalar_tensor_tensor with per-partition gg, residual reload streamed.
"""

import numpy as np

import concourse.bass as bass
import concourse.bacc as bacc
import concourse.mybir as mybir
from concourse.tile import TileContext
from concourse.bass_utils import run_bass_kernel_spmd

F32 = mybir.dt.float32
F32R = mybir.dt.float32r
BF16 = mybir.dt.bfloat16
F16 = mybir.dt.float16
I16 = mybir.dt.int16
I32 = mybir.dt.int32
U32 = mybir.dt.uint32
AX = mybir.AxisListType
OP = mybir.AluOpType
AF = mybir.ActivationFunctionType

C = 256
N = 4096
HEADS = 8
D = 64
PAIRS = 4
INNER = HEADS * D        # 512
NCH = 512
CH = N // NCH            # 8
KEYS = 64                # 8 rows x 8 cols kept per head
EPS = 1e-5


import os
STOP_STAGE = int(os.environ.get("STOP_STAGE", "9"))


def build_program(gval=1.0):
    nc = bacc.Bacc()

    ctx_d = nc.declare_dram_parameter("ctx", [C, N], F32R, False)
    qs_d = nc.declare_dram_parameter("qsrc", [C, N], F32R, False)
    wkvT_d = nc.declare_dram_parameter("wkvT", [C, 2 * INNER], F16, False)
    wqT_d = nc.declare_dram_parameter("wqT", [C, INNER], F16, False)
    woutT_d = nc.declare_dram_parameter("woutT", [INNER, C], BF16, False)
    gg_d = nc.declare_dram_parameter("gg", [C, 1], F32, False)
    ident_d = nc.declare_dram_parameter("identc", [128, 64], F16, False)
    onehot8_d = nc.declare_dram_parameter("onehot8c", [128, 8], F32, False)
    m8f_d = nc.declare_dram_parameter("m8fc", [128, 1], F32, False)
    zsel2_d = nc.declare_dram_parameter("zsel2c", [2, 128], F32, False)
    zsel128_d = nc.declare_dram_parameter("zsel128c", [128, 128], F16, False)
    out_d = nc.declare_dram_parameter("out", [C, N], F32, True)

    with TileContext(nc) as tc:
        with (
            tc.tile_pool(name="const", bufs=1) as constp,
            tc.tile_pool(name="wpool", bufs=1) as wpool,
            tc.tile_pool(name="xin", bufs=3) as xin,
            tc.tile_pool(name="stat", bufs=1) as statp,
            tc.tile_pool(name="xpp", bufs=1) as xpp,
            tc.tile_pool(name="kvq", bufs=1) as kvqp,
            tc.tile_pool(name="pairs", bufs=2) as pairp,
            tc.tile_pool(name="sel", bufs=1) as selp,
            tc.tile_pool(name="attn", bufs=1) as attnp,
            tc.tile_pool(name="ptile", bufs=2) as ptp,
            tc.tile_pool(name="fin", bufs=1) as finp,
            tc.tile_pool(name="psStat", bufs=3, space="PSUM") as psStat,
            tc.tile_pool(name="psMain", bufs=4, space="PSUM") as psMain,
            tc.tile_pool(name="psSmall", bufs=1, space="PSUM") as psSmall,
        ):
            # ------------- constants -------------
            ones128 = constp.tile([128, 128], F32, tag="ones128")
            nc.vector.memset(ones128[:], 1.0)
            ones_r = constp.tile([128, 128], F32R, tag="ones_r")
            nc.scalar.copy(ones_r[:], ones128[:])
            ones16 = constp.tile([128, 128], F16, tag="ones16")
            nc.vector.memset(ones16[:], 1.0)
            ones16b = constp.tile([128, 128], BF16, tag="ones16b")
            nc.vector.memset(ones16b[:], 1.0)
            eps_c = constp.tile([128, 1], F32, tag="eps_c")
            nc.vector.memset(eps_c[:], EPS)
            eps_g = constp.tile([128, 1], F32, tag="eps_g")
            nc.vector.memset(eps_g[:], EPS / (gval * gval))
            # halves8: col 2i ones on partitions 0:64, col 2i+1 on 64:128
            halves8 = constp.tile([128, 8], F16, tag="halves8")
            nc.vector.memset(halves8[:], 0.0)
            for i in range(4):
                nc.vector.memset(halves8[0:64, 2 * i:2 * i + 1], 1.0)
                nc.vector.memset(halves8[64:128, 2 * i + 1:2 * i + 2], 1.0)
            # block-diag ones for per-head-half broadcast sums
            halvesbc16 = constp.tile([128, 128], F16, tag="halvesbc16")
            nc.vector.memset(halvesbc16[:], 0.0)
            nc.vector.memset(halvesbc16[0:64, 0:64], 1.0)
            nc.vector.memset(halvesbc16[64:128, 64:128], 1.0)
            ident16 = constp.tile([128, 64], F16, tag="ident16")
            nc.sync.dma_start(out=ident16[:], in_=ident_d[:])
            zsel2 = constp.tile([2, 128], F32, tag="zsel2")
            nc.sync.dma_start(out=zsel2[:], in_=zsel2_d[:])
            # zsel2f128: selector rows at base 0 and base 64 (matmul requires
            # lhsT/rhs base partitions to match)
            zsel2f128 = constp.tile([128, 128], F16, tag="zsel2f128")
            nc.sync.dma_start(out=zsel2f128[:], in_=zsel128_d[:])
            onehot8 = constp.tile([128, 8], F32, tag="onehot8")
            nc.sync.dma_start(out=onehot8[:], in_=onehot8_d[:])
            m8f = constp.tile([128, 1], F32, tag="m8f")
            nc.sync.dma_start(out=m8f[:], in_=m8f_d[:])

            # ------------- weights -------------
            wkvT = [wpool.tile([128, 2 * INNER], F16, tag=f"wkvT{i}",
                               name=f"wkvT{i}") for i in range(2)]
            wqT = [wpool.tile([128, INNER], F16, tag=f"wqT{i}",
                              name=f"wqT{i}") for i in range(2)]
            for i in range(2):
                nc.sync.dma_start(out=wkvT[i][:], in_=wkvT_d[128 * i:128 * (i + 1), :])
                nc.sync.dma_start(out=wqT[i][:], in_=wqT_d[128 * i:128 * (i + 1), :])
            woutT = [wpool.tile([128, C], BF16, tag=f"woutT{i}",
                                name=f"woutT{i}") for i in range(4)]
            for i in range(4):
                nc.sync.dma_start(out=woutT[i][:], in_=woutT_d[128 * i:128 * (i + 1), :])
            gg = [wpool.tile([128, 1], F32, tag=f"gg{i}", name=f"gg{i}")
                  for i in range(2)]
            for i in range(2):
                nc.sync.dma_start(out=gg[i][:], in_=gg_d[128 * i:128 * (i + 1), :])

            # ------------- phase A: LN (ctx: mu+rstd; qs: mu only) ---------
            # ctx x'' = (x - mu)*rstd (v needs rstd; k's l2norm cancels it);
            # qs x'' = x - mu (q's l2norm cancels any scale). Variance from
            # squares of the centered xc (sqrt fuses 1/C scale + eps bias).
            xpp_t = {
                "ctx": [xpp.tile([128, N], F16, tag=f"xpp_ctx{i}",
                                 name=f"xpp_ctx{i}") for i in range(2)],
                "qs": [xpp.tile([128, N], F16, tag=f"xpp_qs{i}",
                                name=f"xpp_qs{i}") for i in range(2)],
            }

            scopeA = nc.enter_named_scope("phA", False)
            for ch in range(CH):
                sl = slice(ch * NCH, (ch + 1) * NCH)
                xt = [xin.tile([128, NCH], F32R, tag="xt", name="xt", bufs=2)
                      for _ in range(2)]
                xtq = [xin.tile([128, NCH], F32R, tag="xtq", name="xtq", bufs=2)
                       for _ in range(2)]
                for i in range(2):
                    nc.sync.dma_start(out=xt[i][:],
                                      in_=ctx_d[128 * i:128 * (i + 1), sl])
                    nc.scalar.dma_start(out=xtq[i][:],
                                        in_=qs_d[128 * i:128 * (i + 1), sl])
                S_ps = psStat.tile([128, NCH], F32, tag="st")
                nc.tensor.matmul(S_ps[:], lhsT=ones_r[:], rhs=xt[0][:],
                                 start=True, stop=False)
                nc.tensor.matmul(S_ps[:], lhsT=ones_r[:], rhs=xt[1][:],
                                 start=False, stop=True)
                t_mu = statp.tile([128, NCH], F32, tag="t_mu", bufs=2)
                nc.scalar.activation(t_mu[:], S_ps[:], AF.Copy, scale=1.0 / C)
                Sq_ps = psStat.tile([128, NCH], F32, tag="st")
                nc.tensor.matmul(Sq_ps[:], lhsT=ones_r[:], rhs=xtq[0][:],
                                 start=True, stop=False)
                nc.tensor.matmul(Sq_ps[:], lhsT=ones_r[:], rhs=xtq[1][:],
                                 start=False, stop=True)
                t_muq = statp.tile([128, NCH], F32, tag="t_muq", bufs=2)
                nc.scalar.activation(t_muq[:], Sq_ps[:], AF.Copy, scale=1.0 / C)
                # centered ctx (f32, feeds squares + final rstd mult)
                xc = [xin.tile([128, NCH], F32, tag="xc", name="xc", bufs=2)
                      for _ in range(2)]
                for i in range(2):
                    nc.gpsimd.tensor_sub(xc[i][:], xt[i][:].bitcast(F32),
                                         t_mu[:])
                # qs: mean-subtract only
                nc.gpsimd.tensor_sub(xpp_t["qs"][0][:, sl],
                                     xtq[0][:].bitcast(F32), t_muq[:])
                nc.vector.tensor_sub(xpp_t["qs"][1][:, sl],
                                     xtq[1][:].bitcast(F32), t_muq[:])
                # variance of ctx from centered squares
                xsq = [xin.tile([128, NCH], F16, tag="xsq", name="xsq", bufs=2)
                       for _ in range(2)]
                for i in range(2):
                    nc.scalar.activation(xsq[i][:], xc[i][:], AF.Square)
                Q_ps = psStat.tile([128, NCH], F32, tag="st")
                nc.tensor.matmul(Q_ps[:], lhsT=ones16[:], rhs=xsq[0][:],
                                 start=True, stop=False)
                nc.tensor.matmul(Q_ps[:], lhsT=ones16[:], rhs=xsq[1][:],
                                 start=False, stop=True)
                se = statp.tile([128, NCH], F32, tag="se", bufs=2)
                nc.scalar.activation(se[:], Q_ps[:], AF.Sqrt, bias=eps_c[:],
                                     scale=1.0 / C)
                rstd = statp.tile([128, NCH], F32, tag="rstd", bufs=2)
                nc.vector.reciprocal_approx_fast(out=rstd[:], in_=se[:])
                for i in range(2):
                    nc.vector.tensor_tensor(out=xpp_t["ctx"][i][:, sl],
                                            in0=xc[i][:], in1=rstd[:],
                                            op=OP.mult)
            nc.leave_named_scope("phA", scopeA[0], False)

            # ------------- phase P: proj + norms + probe + topk + gather ---
            il_t, qh_t, ksel_t, kbd_t, vbd_t = {}, {}, {}, {}, {}
            kabs_r_t, kabsc8_t = {}, {}
            ao16 = [attnp.tile([128, N], BF16, tag=f"ao{p}", name=f"ao{p}")
                    for p in range(PAIRS)]

            def alloc_pair(p):
                if p not in il_t:
                    il_t[p] = kvqp.tile([128, 2 * N], F16, tag="il", bufs=1,
                                        name=f"il{p}")
                    qh_t[p] = kvqp.tile([128, N], F16, tag="qh", bufs=4,
                                        name=f"qh{p}")

            def do_b1q(p, chunks):
                alloc_pair(p)
                qh = qh_t[p]
                for ch in chunks:
                    sl = slice(ch * NCH, (ch + 1) * NCH)
                    qps = psMain.tile([128, NCH], F32, tag="m")
                    nc.tensor.matmul(qps[:], lhsT=wqT[0][:, 128 * p:128 * (p + 1)],
                                     rhs=xpp_t["qs"][0][:, sl], start=True, stop=False)
                    nc.tensor.matmul(qps[:], lhsT=wqT[1][:, 128 * p:128 * (p + 1)],
                                     rhs=xpp_t["qs"][1][:, sl], start=False, stop=True)
                    q2c = kvqp.tile([128, NCH], F16, tag="q2c", bufs=1)
                    nc.scalar.activation(q2c[:], qps[:], AF.Square)
                    rqps = psStat.tile([128, NCH], F32, tag="st")
                    nc.tensor.matmul(rqps[:], lhsT=halvesbc16[:], rhs=q2c[:],
                                     start=True, stop=True)
                    seq2 = statp.tile([128, NCH], F32, tag="se_", bufs=2)
                    nc.scalar.activation(seq2[:], rqps[:], AF.Sqrt)
                    rbq = statp.tile([128, NCH], F32, tag="rb_", bufs=2)
                    nc.vector.reciprocal_approx_fast(out=rbq[:], in_=seq2[:])
                    nc.vector.tensor_tensor(out=qh[:, sl], in0=qps[:],
                                            in1=rbq[:], op=OP.mult)

            def do_b1kv(p, chunks):
                alloc_pair(p)
                il = il_t[p]
                for ch in chunks:
                    sl = slice(ch * NCH, (ch + 1) * NCH)
                    # --- k projection + l2 factor + il write ---
                    kps = psMain.tile([128, NCH], F32, tag="m")
                    nc.tensor.matmul(kps[:], lhsT=wkvT[0][:, 128 * p:128 * (p + 1)],
                                     rhs=xpp_t["ctx"][0][:, sl], start=True, stop=False)
                    nc.tensor.matmul(kps[:], lhsT=wkvT[1][:, 128 * p:128 * (p + 1)],
                                     rhs=xpp_t["ctx"][1][:, sl], start=False, stop=True)
                    k16c = kvqp.tile([128, NCH], F16, tag="k16c", bufs=2)
                    nc.scalar.copy(k16c[:], kps[:])
                    k2c = kvqp.tile([128, NCH], F16, tag="k2c", bufs=2)
                    nc.vector.tensor_mul(k2c[:], k16c[:], k16c[:])
                    rkps = psStat.tile([128, NCH], F32, tag="st")
                    nc.tensor.matmul(rkps[:], lhsT=halvesbc16[:], rhs=k2c[:],
                                     start=True, stop=True)
                    sek = statp.tile([128, NCH], F32, tag="se_", bufs=2)
                    nc.scalar.activation(sek[:], rkps[:], AF.Sqrt)
                    rbk = statp.tile([128, NCH], F32, tag="rb_", bufs=2)
                    nc.vector.reciprocal_approx_fast(out=rbk[:], in_=sek[:])
                    nc.gpsimd.tensor_tensor(out=il[:, 2 * sl.start:2 * sl.stop:2],
                                            in0=k16c[:], in1=rbk[:], op=OP.mult)
                    # --- v projection (rstd already folded into ctx x'') ---
                    vps = psMain.tile([128, NCH], F32, tag="m")
                    vo = INNER + 128 * p
                    nc.tensor.matmul(vps[:], lhsT=wkvT[0][:, vo:vo + 128],
                                     rhs=xpp_t["ctx"][0][:, sl], start=True, stop=False)
                    nc.tensor.matmul(vps[:], lhsT=wkvT[1][:, vo:vo + 128],
                                     rhs=xpp_t["ctx"][1][:, sl], start=False, stop=True)
                    nc.scalar.copy(il[:, 2 * sl.start + 1:2 * sl.stop:2], vps[:])

            def do_b2(p):
                il, qh = il_t[p], qh_t[p]
                # --- segmented |khat| sums + q_probe + scores + topk ---
                il4 = il[:].rearrange("p (h w d) -> p h w d", h=64, w=64, d=2)
                kabs_r = pairp.tile([128, 64], F32, tag="kabsr")
                nc.vector.tensor_reduce(out=kabs_r[:], in_=il4[:, :, :, 0],
                                        axis=AX.X, op=OP.add, apply_absolute_value=True)
                il4c = il[:].rearrange("p (h w d) -> p w h d", h=64, w=64, d=2)
                kabs_c = pairp.tile([128, 64], F32, tag="kabsc")
                nc.vector.tensor_reduce(out=kabs_c[:], in_=il4c[:, :, :, 0],
                                        axis=AX.X, op=OP.add, apply_absolute_value=True)
                qp = pairp.tile([128, 1], F32, tag="qp")
                nc.vector.tensor_reduce(out=qp[:], in_=qh[:], axis=AX.X, op=OP.add)
                qp2 = pairp.tile([128, 2], F32, tag="qp2")
                nc.vector.memset(qp2[:], 0.0)
                nc.vector.tensor_copy(out=qp2[0:64, 0:1], in_=qp[0:64, :])
                nc.vector.tensor_copy(out=qp2[64:128, 1:2], in_=qp[64:128, :])
                sc_r = pairp.tile([2, 64], F32, tag="scr")
                sc_ps = psSmall.tile([2, 64], F32, tag="s")
                nc.tensor.matmul(sc_ps[:], lhsT=qp2[:], rhs=kabs_r[:],
                                 start=True, stop=True)
                nc.scalar.copy(sc_r[:], sc_ps[:])
                sc_c = pairp.tile([2, 64], F32, tag="scc")
                sc_ps2 = psSmall.tile([2, 64], F32, tag="s")
                nc.tensor.matmul(sc_ps2[:], lhsT=qp2[:], rhs=kabs_c[:],
                                 start=True, stop=True)
                nc.scalar.copy(sc_c[:], sc_ps2[:])
                mx = pairp.tile([2, 8], F32, tag="mx")
                idx_r = pairp.tile([2, 8], U32, tag="idxr")
                nc.vector.max(out=mx[:], in_=sc_r[:])
                nc.vector.max_index(out=idx_r[:], in_max=mx[:], in_values=sc_r[:])
                mxc = pairp.tile([2, 8], F32, tag="mxc")
                idx_c = pairp.tile([2, 8], U32, tag="idxc")
                nc.vector.max(out=mxc[:], in_=sc_c[:])
                nc.vector.max_index(out=idx_c[:], in_max=mxc[:], in_values=sc_c[:])
                idxr_f = pairp.tile([2, 8], F32, tag="idxrf")
                nc.vector.tensor_copy(out=idxr_f[:], in_=idx_r[:])
                idxc_f = pairp.tile([2, 8], F32, tag="idxcf")
                nc.vector.tensor_copy(out=idxc_f[:], in_=idx_c[:])
                # broadcast idx rows to all partitions by head half
                rbc_ps = psSmall.tile([128, 8], F32, tag="s")
                nc.tensor.matmul(rbc_ps[:], lhsT=zsel2[:], rhs=idxr_f[:],
                                 start=True, stop=True)
                rbc = pairp.tile([128, 8], F32, tag="rbc")
                nc.scalar.copy(rbc[:], rbc_ps[:])
                cbc_ps = psSmall.tile([128, 8], F32, tag="s")
                nc.tensor.matmul(cbc_ps[:], lhsT=zsel2[:], rhs=idxc_f[:],
                                 start=True, stop=True)
                cbc = pairp.tile([128, 8], F32, tag="cbc")
                nc.scalar.copy(cbc[:], cbc_ps[:])
                # Bcol[p] = idx_c[h(p), p%8]
                junk8 = pairp.tile([128, 8], F32, tag="junk8")
                nc.vector.tensor_mul(junk8[:], cbc[:], onehot8[:])
                Bcol = pairp.tile([128, 1], F32, tag="Bcol")
                nc.vector.tensor_reduce(out=Bcol[:], in_=junk8[:], axis=AX.X,
                                        op=OP.add)
                # wr[p, s] = idx_r[h(p), 2s + ((p>>3)&1)]
                wdiff = pairp.tile([128, 4], F32, tag="wdiff")
                nc.vector.tensor_sub(wdiff[:], rbc[:, 1:8:2], rbc[:, 0:8:2])
                wsel = pairp.tile([128, 4], F32, tag="wsel")
                nc.vector.tensor_scalar(wsel[:], wdiff[:], m8f[:], scalar2=None,
                                        op0=OP.mult)
                wr = pairp.tile([128, 4], F32, tag="wr")
                nc.vector.tensor_add(wr[:], wsel[:], rbc[:, 0:8:2])
                posfw = pairp.tile([128, 4], F32, tag="posfw")
                nc.vector.scalar_tensor_tensor(out=posfw[:], in0=wr[:], scalar=64.0,
                                               in1=Bcol[:].to_broadcast([128, 4]),
                                               op0=OP.mult, op1=OP.add)
                widx32 = pairp.tile([128, 4], I32, tag="widx32")
                nc.vector.tensor_copy(out=widx32[:], in_=posfw[:])
                widx = pairp.tile([128, 4], I16, tag="widx")
                nc.vector.tensor_copy(out=widx[:], in_=widx32[:])
                # --- gather ---
                ksel_il = selp.tile([128, 128], F16, tag="kselil", bufs=2,
                                    name=f"ksel{p}")
                nc.gpsimd.ap_gather(
                    out_ap=ksel_il[:].rearrange("p (k d) -> p k d", d=2),
                    in_ap=il[:].rearrange("p (n d) -> p n d", d=2),
                    idxs_ap=widx[:],
                    channels=128, num_elems=N, d=2, num_idxs=KEYS)
                ksel_t[p] = ksel_il

            def do_extract(p):
                ksel_il = ksel_t[p]
                kbd = selp.tile([128, 128], F16, tag="kbd", bufs=4, name=f"kbd{p}")
                nc.vector.memset(kbd[:], 0.0)
                nc.vector.tensor_copy(out=kbd[0:64, 0:64], in_=ksel_il[0:64, 0:128:2])
                nc.vector.tensor_copy(out=kbd[64:128, 64:128],
                                      in_=ksel_il[64:128, 0:128:2])
                vbd = selp.tile([128, 128], F16, tag="vbd", bufs=4, name=f"vbd{p}")
                nc.vector.memset(vbd[:], 0.0)
                for h in range(2):
                    o = 64 * h
                    tps = psSmall.tile([64, 64], F16, tag="s")
                    nc.tensor.transpose(out=tps[:], in_=ksel_il[o:o + 64, 1:128:2],
                                        identity=ident16[o:o + 64, :])
                    nc.scalar.copy(vbd[o:o + 64, o:o + 64], tps[:])
                kbd_t[p], vbd_t[p] = kbd, vbd

            # il has a single buffer: gather(p) must be emitted before any
            # il(p+1) writes (in-order gpsimd queue keeps this deadlock-free).
            scopeP = nc.enter_named_scope("phP", False)
            if STOP_STAGE >= 2:
                for p in range(PAIRS):
                    do_b1q(p, range(CH))
                    do_b1kv(p, range(CH))
                    if STOP_STAGE >= 3:
                        do_b2(p)
                        do_extract(p)
            nc.leave_named_scope("phP", scopeP[0], False)

            # ------------- phase X: attention (Exp table) -------------
            scopeX = nc.enter_named_scope("phX", False)

            def do_b3(p):
                kbd, vbd, qh = kbd_t[p], vbd_t[p], qh_t[p]
                for g in range(4):          # 2-chunk groups share one Z bank
                    # chunk 2g -> zall[0:2], chunk 2g+1 -> zall[64:66]
                    zall = psSmall.tile([128, NCH], F32, tag="s",
                                        name=f"zall{p}{g}")
                    pts = []
                    for i in range(2):
                        ch = 2 * g + i
                        sl = slice(ch * NCH, (ch + 1) * NCH)
                        sps = psMain.tile([128, NCH], F32, tag="m")
                        nc.tensor.matmul(sps[:], lhsT=kbd[:], rhs=qh[:, sl],
                                         start=True, stop=True)
                        pt = ptp.tile([128, NCH], F16, tag="pT", bufs=4)
                        nc.scalar.activation(pt[:], sps[:], AF.Exp)
                        nc.tensor.matmul(zall[64 * i:64 * i + 2, :],
                                         lhsT=halves8[:, 0:2],
                                         rhs=pt[:], start=True, stop=True)
                        pts.append(pt)
                    zinv = ptp.tile([128, NCH], F32, tag="zinv", bufs=2)
                    nc.vector.reciprocal_approx_fast(out=zinv[0:66, :],
                                                     in_=zall[0:66, :])
                    zinv16 = ptp.tile([128, NCH], F16, tag="zinv16", bufs=2)
                    nc.scalar.copy(zinv16[0:66, :], zinv[0:66, :])
                    for i in range(2):
                        ch = 2 * g + i
                        sl = slice(ch * NCH, (ch + 1) * NCH)
                        zb = psStat.tile([128, NCH], F32, tag="st")
                        nc.tensor.matmul(zb[:], lhsT=zsel2f128[64 * i:64 * i + 2, :],
                                         rhs=zinv16[64 * i:64 * i + 2, :],
                                         start=True, stop=True)
                        ph16 = ptp.tile([128, NCH], F16, tag="ph16", bufs=2)
                        nc.vector.tensor_tensor(out=ph16[:], in0=pts[i][:],
                                                in1=zb[:], op=OP.mult)
                        pvs = psMain.tile([128, NCH], F32, tag="m")
                        nc.tensor.matmul(pvs[:], lhsT=vbd[:], rhs=ph16[:],
                                         start=True, stop=True)
                        if ch % 2 == 0:
                            nc.scalar.copy(ao16[p][:, sl], pvs[:])
                        else:
                            nc.vector.tensor_copy(out=ao16[p][:, sl], in_=pvs[:])

            if STOP_STAGE >= 4:
                for p in range(PAIRS):
                    do_b3(p)
            nc.leave_named_scope("phX", scopeX[0], False)

            # ------------- tail: out-proj + out-LN + residual -------------
            scopeT = nc.enter_named_scope("phT", False)
            for ch in range(CH if STOP_STAGE >= 5 else 0):
                sl = slice(ch * NCH, (ch + 1) * NCH)
                qs_t = [finp.tile([128, NCH], F32, tag=f"qs_t{i}", bufs=1,
                                  name=f"qs_t{i}_{ch}") for i in range(2)]
                nc.sync.dma_start(out=qs_t[0][:],
                                  in_=qs_d[0:128, sl].bitcast(F32))
                nc.scalar.dma_start(out=qs_t[1][:],
                                    in_=qs_d[128:256, sl].bitcast(F32))
                y16 = [finp.tile([128, NCH], BF16, tag="y16", name=f"y16_{ch}_{i}",
                                 bufs=2) for i in range(2)]
                for i in range(2):
                    yps = psMain.tile([128, NCH], F32, tag="m")
                    for p in range(PAIRS):
                        nc.tensor.matmul(yps[:],
                                         lhsT=woutT[p][:, 128 * i:128 * (i + 1)],
                                         rhs=ao16[p][:, sl], start=(p == 0),
                                         stop=(p == 3))
                    nc.scalar.copy(y16[i][:, :], yps[:])
                S_ps = psStat.tile([128, NCH], F32, tag="st")
                nc.tensor.matmul(S_ps[:], lhsT=ones16b[:], rhs=y16[0][:],
                                 start=True, stop=False)
                nc.tensor.matmul(S_ps[:], lhsT=ones16b[:], rhs=y16[1][:],
                                 start=False, stop=True)
                t_mu = finp.tile([128, NCH], F32, tag="ft_mu", bufs=1)
                nc.scalar.activation(t_mu[:], S_ps[:], AF.Copy, scale=1.0 / C)
                ty = [finp.tile([128, NCH], BF16, tag="fty", name=f"ty{ch}_{i}",
                                bufs=2) for i in range(2)]
                nc.vector.tensor_sub(ty[0][:], y16[0][:], t_mu[:])
                nc.vector.tensor_sub(ty[1][:], y16[1][:], t_mu[:])
                y2 = [finp.tile([128, NCH], F16, tag="fy2", bufs=1,
                                name=f"y2_{ch}_{i}") for i in range(2)]
                for i in range(2):
                    nc.vector.tensor_mul(y2[i][:], ty[i][:], ty[i][:])
                Q_ps = psStat.tile([128, NCH], F32, tag="st")
                nc.tensor.matmul(Q_ps[:], lhsT=ones16[:], rhs=y2[0][:],
                                 start=True, stop=False)
                nc.tensor.matmul(Q_ps[:], lhsT=ones16[:], rhs=y2[1][:],
                                 start=False, stop=True)
                # sqrt((1/(C*g^2))*x + eps/g^2) = sqrt(x/C+eps)/g, so the
                # reciprocal directly yields g*rstd (g = uniform gamma*on_g).
                se = finp.tile([128, NCH], F32, tag="fse", bufs=1)
                nc.scalar.activation(se[:], Q_ps[:], AF.Sqrt, bias=eps_g[:],
                                     scale=1.0 / (C * gval * gval))
                rstd_o = finp.tile([128, NCH], F32, tag="frstd", bufs=1)
                nc.vector.reciprocal_approx_fast(out=rstd_o[:], in_=se[:])
                for i in range(2):
                    tz = finp.tile([128, NCH], F32, tag="ftz", name=f"tz{ch}_{i}",
                                   bufs=2)
                    nc.vector.tensor_mul(tz[:], ty[i][:], rstd_o[:])
                    ot = finp.tile([128, NCH], F32, tag="fot", name=f"ot{ch}_{i}",
                                   bufs=2)
                    nc.gpsimd.tensor_add(ot[:], tz[:], qs_t[i][:])
                    if i == 0:
                        nc.sync.dma_start(out=out_d[0:128, sl], in_=ot[:])
                    else:
                        nc.scalar.dma_start(out=out_d[128:256, sl], in_=ot[:])
            if STOP_STAGE < 5:
                for ch in range(CH):
                    sl = slice(ch * NCH, (ch + 1) * NCH)
                    for i in range(2):
                        dummy = finp.tile([128, NCH], F32, tag="fot",
                                          name=f"dummy{ch}_{i}", bufs=2)
                        nc.vector.memset(dummy[:], 0.0)
                        nc.sync.dma_start(out=out_d[128 * i:128 * (i + 1), sl],
                                          in_=dummy[:])
            nc.leave_named_scope("phT", scopeT[0], False)
    nc.finalize()
    return nc


_CACHE = {}


def prepare(inputs):
    """Build (nc, in_maps) from full unsharded inputs."""
    qsrc = np.asarray(inputs["query_source"], np.float32)
    ctx = np.asarray(inputs["context"], np.float32)
    cn_g = np.asarray(inputs["cn_g"], np.float32).reshape(C)
    cn_b = np.asarray(inputs["cn_b"], np.float32).reshape(C)
    qn_g = np.asarray(inputs["qn_g"], np.float32).reshape(C)
    qn_b = np.asarray(inputs["qn_b"], np.float32).reshape(C)
    on_g = np.asarray(inputs["on_g"], np.float32).reshape(C)
    on_b = np.asarray(inputs["on_b"], np.float32).reshape(C)
    w_kv = np.asarray(inputs["w_kv"], np.float32)
    w_q = np.asarray(inputs["w_q"], np.float32)
    w_out = np.asarray(inputs["w_out"], np.float32)
    gamma = float(np.asarray(inputs["gamma"], np.float32).reshape(()))

    assert np.abs(cn_b).max() == 0 and np.abs(qn_b).max() == 0 and \
        np.abs(on_b).max() == 0, "nonzero LN bias not implemented"

    import ml_dtypes
    bf16 = ml_dtypes.bfloat16
    wkvT = np.ascontiguousarray((w_kv * cn_g[None, :]).T).astype(np.float16)
    wqT = np.ascontiguousarray((w_q * qn_g[None, :]).T).astype(np.float16)
    woutT = np.ascontiguousarray(w_out.T).astype(bf16)
    gg = np.ascontiguousarray((gamma * on_g).reshape(C, 1), np.float32)

    p_idx = np.arange(128)
    identc = np.zeros((128, 64), np.float16)
    identc[p_idx, p_idx % 64] = 1.0
    onehot8c = (p_idx[:, None] % 8 == np.arange(8)[None, :]).astype(np.float32)
    m8fc = (((p_idx >> 3) & 1).astype(np.float32)).reshape(128, 1)
    zsel2c = (np.arange(128)[None, :] // 64 ==
              np.arange(2)[:, None]).astype(np.float32)
    zsel128c = np.zeros((128, 128), np.float16)
    for b in (0, 64):
        zsel128c[b, 0:64] = 1.0
        zsel128c[b + 1, 64:128] = 1.0

    gv = float(gg[0, 0])
    assert np.allclose(gg, gv), "nonuniform out-LN gain not implemented"
    if _CACHE.get("gval") != gv:
        _CACHE["nc"] = build_program(gv)
        _CACHE["gval"] = gv
    nc = _CACHE["nc"]

    B = qsrc.shape[0]
    in_maps = []
    for b in range(B):
        in_maps.append({
            "ctx": np.ascontiguousarray(ctx[b].reshape(C, N)),
            "qsrc": np.ascontiguousarray(qsrc[b].reshape(C, N)),
            "wkvT": wkvT,
            "wqT": wqT,
            "woutT": woutT,
            "gg": gg,
            "identc": identc,
            "onehot8c": onehot8c,
            "m8fc": m8fc,
            "zsel2c": zsel2c,
            "zsel128c": zsel128c,
        })
    return nc, in_maps


def kernel(**inputs):
    nc, in_maps = prepare(inputs)
    res = run_bass_kernel_spmd(nc, in_maps, core_ids=list(range(8)))
    outs = [np.asarray(r["out"], np.float32).reshape(1, C, 64, 64)
            for r in res.results]
    return np.concatenate(outs, axis=0)
